# revision 1
# baseline (speedup 1.0000x reference)
"""Sequence-parallel self-attention kernel for 8 TRN2 NeuronCores.

Reference computation (N=8192, D=256, fp32):
    q = x @ WQ; k = x @ WK; v = x @ WV
    out = softmax(q @ k.T) @ v

Sharding: q-rows are split across 8 cores (1024 rows each); x is replicated
(host passes it pre-transposed as xT plus natural-layout x), so no
collectives are needed.

Per-core algebra (everything stays transposed so softmax's k-reduction is a
partition-axis ones-matmul and no on-chip transposes are needed):
    qT = WQ.T @ xT_local                      [256, 1024]
    M  = WK @ qT        (lhsT = WK.T)         [256, 1024]
    per k-chunk c (64 chunks of 128):
      scoresT = x_c @ M                       [128, 1024]   (= (q @ k.T).T chunk)
      expT    = exp(scoresT - 15)             (constant shift cancels in softmax)
      sums   += ones[128,1].T @ expT          [1, 1024]     (softmax denominator)
      UT     += x_c.T @ expT                  [256, 1024]   (= (attn_unnorm @ x).T)
    UTn  = UT * broadcast(1/sums)
    outT = WV.T @ UTn                         [256, 1024]   (= out.T, host transposes)

All matmuls run as float32r (full PE rate at free-dim >= 256, vs 4x slower
fp32). Every tensor feeding a matmul is declared float32r end-to-end (DRAM
inputs included) — the BIR verifier requires producers of fp32r-consumed
data to emit fp32r; numpy still sees plain float32 bytes.

PSUM budget (8 banks): UT 2x[128,1024]=4, sums 2x[1,512]=2, scoresT
double-buffer 2x[128,512]=2. Tail tiles reuse the same pool slots.
"""

import numpy as np

N, D, P = 8192, 256, 8
NL = N // P          # 1024 q-rows per core
KC = 128             # k-chunk size (contraction tile)
NCHUNK = N // KC     # 64
SB = 8               # k-chunks per DMA superblock
EXP_SHIFT = -15.0    # exp(s - 15): keeps ACT exp-table args in a good range

_CACHE = {}


def _build():
    import concourse.bacc as bacc
    import concourse.mybir as mybir
    import concourse.tile as tile

    f32 = mybir.dt.float32
    f32r = mybir.dt.float32r
    EXP = mybir.ActivationFunctionType.Exp

    nc = bacc.Bacc("TRN2", target_bir_lowering=False, debug=False,
                   enable_asserts=False)

    xT = nc.dram_tensor("xT", [D, N], f32r, kind="ExternalInput").ap()
    xn = nc.dram_tensor("xn", [N, D], f32r, kind="ExternalInput").ap()
    xTl = nc.dram_tensor("xTl", [D, NL], f32r, kind="ExternalInput").ap()
    wq = nc.dram_tensor("wq", [D, D], f32r, kind="ExternalInput").ap()
    wkt = nc.dram_tensor("wkt", [D, D], f32r, kind="ExternalInput").ap()
    wv = nc.dram_tensor("wv", [D, D], f32r, kind="ExternalInput").ap()
    onesd = nc.dram_tensor("onesd", [128, 128], f32r, kind="ExternalInput").ap()
    outT = nc.dram_tensor("outT", [D, NL], f32, kind="ExternalOutput").ap()

    with tile.TileContext(nc) as tc:
        with (
            tc.tile_pool(name="const", bufs=1) as cpool,
            tc.tile_pool(name="proj", bufs=1) as ppool,
            tc.tile_pool(name="xts", bufs=4) as xtpool,
            tc.tile_pool(name="xns", bufs=4) as xnpool,
            tc.tile_pool(name="expt", bufs=8) as epool,
            tc.tile_pool(name="tail", bufs=1) as tpool,
            tc.tile_pool(name="ps_scores", bufs=2, space="PSUM") as ps_s,
            tc.tile_pool(name="ps_ut", bufs=1, space="PSUM") as ps_ut,
            tc.tile_pool(name="ps_sums", bufs=1, space="PSUM") as ps_sum,
        ):
            # ---- constants / weights ----
            wq_t = [cpool.tile([128, D], f32r, tag=f"wq{h}", name=f"wq{h}") for h in range(2)]
            wkt_t = [cpool.tile([128, D], f32r, tag=f"wkt{h}", name=f"wkt{h}") for h in range(2)]
            wv_t = [cpool.tile([128, D], f32r, tag=f"wv{h}", name=f"wv{h}") for h in range(2)]
            xTl_t = [cpool.tile([128, NL], f32r, tag=f"xtl{h}", name=f"xtl{h}") for h in range(2)]
            ones_col = cpool.tile([128, 1], f32r, tag="ones_col", name="ones_col")
            ones_row = cpool.tile([1, 128], f32r, tag="ones_row", name="ones_row")
            bias_t = cpool.tile([128, 1], f32, tag="bias_t", name="bias_t")
            for h in range(2):
                nc.sync.dma_start(wq_t[h][:], wq[h * 128:(h + 1) * 128, :])
                nc.sync.dma_start(wkt_t[h][:], wkt[h * 128:(h + 1) * 128, :])
                nc.sync.dma_start(wv_t[h][:], wv[h * 128:(h + 1) * 128, :])
                nc.sync.dma_start(xTl_t[h][:], xTl[h * 128:(h + 1) * 128, :])
            nc.sync.dma_start(ones_col[:], onesd[:, 0:1])
            nc.sync.dma_start(ones_row[:], onesd[0:1, :])
            nc.vector.memset(bias_t[:], EXP_SHIFT)

            # ---- qT = WQ.T @ xT_local ; M = WK @ qT ----
            qT_t = [ppool.tile([128, NL], f32r, tag=f"qt{h}", name=f"qt{h}") for h in range(2)]
            m_t = [ppool.tile([128, NL], f32r, tag=f"m{h}", name=f"m{h}") for h in range(2)]
            for dst, lhs in ((qT_t, wq_t), (m_t, wkt_t)):
                src = xTl_t if dst is qT_t else qT_t
                for mh in range(2):
                    for nh in range(2):
                        pp = ps_s.tile([128, 512], f32, tag="scores", name="scores")
                        for kp in range(2):
                            nc.tensor.matmul(
                                pp[:],
                                lhs[kp][:, mh * 128:(mh + 1) * 128],
                                src[kp][:, nh * 512:(nh + 1) * 512],
                                start=(kp == 0), stop=(kp == 1),
                            )
                        nc.vector.tensor_copy(
                            dst[mh][:, nh * 512:(nh + 1) * 512], pp[:])

            # ---- persistent accumulators ----
            ut_ps = [ps_ut.tile([128, NL], f32, tag=f"ut{h}", name=f"ut{h}") for h in range(2)]
            sums_ps = [ps_sum.tile([1, 512], f32, tag=f"sums{h}", name=f"sums{h}")
                       for h in range(2)]

            # ---- main k-loop ----
            for sb in range(N // (KC * SB)):
                xt_t = [xtpool.tile([128, KC * SB], f32r, tag=f"xt{h}", name=f"xt{h}")
                        for h in range(2)]
                for h in range(2):
                    nc.sync.dma_start(
                        xt_t[h][:],
                        xT[h * 128:(h + 1) * 128,
                           sb * KC * SB:(sb + 1) * KC * SB])
                xn_t = xnpool.tile([128, SB, D], f32r, tag="xn", name="xn")
                nc.sync.dma_start(
                    xn_t[:],
                    xn[sb * KC * SB:(sb + 1) * KC * SB, :]
                    .rearrange("(a p) d -> p a d", p=128))

                for j in range(SB):
                    c = sb * SB + j
                    first, last = (c == 0), (c == NCHUNK - 1)
                    exps = []
                    for qh in range(2):
                        sp = ps_s.tile([128, 512], f32, tag="scores", name="scores")
                        for kp in range(2):
                            nc.tensor.matmul(
                                sp[:],
                                xt_t[kp][:, j * KC:(j + 1) * KC],
                                m_t[kp][:, qh * 512:(qh + 1) * 512],
                                start=(kp == 0), stop=(kp == 1),
                            )
                        et = epool.tile([128, 512], f32r, tag="expt", name="expt")
                        nc.scalar.activation(et[:], sp[:], EXP, bias=bias_t[:])
                        exps.append(et)
                    for qh in range(2):
                        et = exps[qh]
                        nc.tensor.matmul(
                            sums_ps[qh][:], ones_col[:], et[:],
                            start=first, stop=last)
                        for dh in range(2):
                            nc.tensor.matmul(
                                ut_ps[dh][:, qh * 512:(qh + 1) * 512],
                                xn_t[:, j, dh * 128:(dh + 1) * 128],
                                et[:],
                                start=first, stop=last)

            # ---- tail: softmax normalize + WV projection ----
            sums_sb = tpool.tile([1, NL], f32, tag="sums_sb", name="sums_sb")
            for qh in range(2):
                nc.vector.tensor_copy(
                    sums_sb[:, qh * 512:(qh + 1) * 512], sums_ps[qh][:])
            recip_sb = tpool.tile([1, NL], f32r, tag="recip_sb", name="recip_sb")
            with nc.allow_low_precision(reason="f32r is 4-byte, same mantissa path"):
                nc.vector.reciprocal(recip_sb[:], sums_sb[:])

            rb_sb = tpool.tile([128, NL], f32, tag="rb_sb", name="rb_sb")
            for qh in range(2):
                rp = ps_s.tile([128, 512], f32, tag="scores", name="scores")
                nc.tensor.matmul(
                    rp[:], ones_row[:],
                    recip_sb[:, qh * 512:(qh + 1) * 512],
                    start=True, stop=True)
                nc.vector.tensor_copy(rb_sb[:, qh * 512:(qh + 1) * 512], rp[:])

            utn_sb = [tpool.tile([128, NL], f32r, tag=f"utn{h}", name=f"utn{h}")
                      for h in range(2)]
            for dh in range(2):
                nc.vector.tensor_mul(utn_sb[dh][:], ut_ps[dh][:], rb_sb[:])

            o_sb = [tpool.tile([128, NL], f32, tag=f"osb{h}", name=f"osb{h}") for h in range(2)]
            for mh in range(2):
                op = ps_ut.tile([128, NL], f32, tag=f"ut{mh}", name=f"ut{mh}")
                for nh in range(2):
                    for kp in range(2):
                        nc.tensor.matmul(
                            op[:, nh * 512:(nh + 1) * 512],
                            wv_t[kp][:, mh * 128:(mh + 1) * 128],
                            utn_sb[kp][:, nh * 512:(nh + 1) * 512],
                            start=(kp == 0), stop=(kp == 1),
                        )
                nc.vector.tensor_copy(o_sb[mh][:], op[:])
                nc.sync.dma_start(outT[mh * 128:(mh + 1) * 128, :], o_sb[mh][:])

    nc.compile()
    return nc


def _get_nc():
    if "nc" not in _CACHE:
        _CACHE["nc"] = _build()
    return _CACHE["nc"]


def kernel(input, WQ, WK, WV):
    from concourse import bass_utils

    x = np.ascontiguousarray(input, dtype=np.float32)
    xT = np.ascontiguousarray(x.T)
    wq = np.ascontiguousarray(WQ, dtype=np.float32)
    wkt = np.ascontiguousarray(np.asarray(WK, dtype=np.float32).T)
    wv = np.ascontiguousarray(WV, dtype=np.float32)

    nc = _get_nc()
    in_maps = []
    for c in range(P):
        in_maps.append({
            "xT": xT,
            "xn": x,
            "xTl": np.ascontiguousarray(xT[:, c * NL:(c + 1) * NL]),
            "wq": wq,
            "wkt": wkt,
            "wv": wv,
            "onesd": np.ones((128, 128), dtype=np.float32),
        })
    res = bass_utils.run_bass_kernel_spmd(nc, in_maps, core_ids=list(range(P)))
    out = np.empty((N, D), dtype=np.float32)
    for c in range(P):
        out[c * NL:(c + 1) * NL, :] = res.results[c]["outT"].T
    return out



# revision 4
# speedup vs baseline: 7.7623x; 7.7623x over previous
"""Sequence-parallel self-attention kernel for 8 TRN2 NeuronCores.

Reference computation (N=8192, D=256, fp32):
    q = x @ WQ; k = x @ WK; v = x @ WV
    out = softmax(q @ k.T) @ v

Host->device traffic is the wall-clock bottleneck (axon tunnel ~35 MB/s), so
each core receives ONLY its own fp16 shard plus a 1/8 slice of the packed
weights (~0.55 MB/core instead of 17.8 MB/core replicated), and the full x is
reconstructed on-device with AllGathers over NeuronLink (~14 us each):

  per core c:
    xs_h  [1024, 256] fp16   own x rows (natural layout)
    w_h   [96, 256]   fp16   rows c*96..(c+1)*96 of packed [WQ; WK.T; WV]
  on device:
    AG#1: cast(xs_h)->f32r, gather -> xg  [8192, 256]   (natural x)
    AG#2: XBAR dma-transpose(xs_h)->f32r, gather -> xgT [2048, 1024]
          (8 stacked [256,1024] per-core transposed shards)
    AG#3: gather w_h -> w_all [768, 256] fp16, cast -> f32r weight tiles

Per-core algebra (identical to the proven replicated-input kernel; everything
stays transposed so softmax's k-reduction is a partition-axis ones-matmul):
    qT = WQ.T @ xT_local                      [256, 1024]
    M  = WK @ qT        (lhsT = WK.T)         [256, 1024]
    per k-chunk c (64 chunks of 128):
      scoresT = x_c @ M                       [128, 1024]   (= (q @ k.T).T chunk)
      expT    = exp(scoresT - 15)             (constant shift cancels in softmax)
      sums   += ones[128,1].T @ expT          [1, 1024]     (softmax denominator)
      UT     += x_c.T @ expT                  [256, 1024]   (= (attn_unnorm @ x).T)
    UTn  = UT * broadcast(1/sums)
    outT = WV.T @ UTn                         [256, 1024]   (fp16 out, host
                                                             transposes+upcasts)

All matmuls run as float32r (full PE rate at free-dim >= 256). fp16 transport
adds ~3e-3 rel err on top of f32r's ~1.5e-3 -- well under the 2e-2 gate.
"""

import numpy as np

N, D, P = 8192, 256, 8
NL = N // P          # 1024 q-rows per core
KC = 128             # k-chunk size (contraction tile)
NCHUNK = N // KC     # 64
SB = 8               # k-chunks per DMA superblock
WSH = 3 * D // P     # 96 packed-weight rows per core
EXP_SHIFT = -15.0    # exp(s - 15): keeps ACT exp-table args in a good range

_CACHE = {}


def _build():
    import concourse.bacc as bacc
    import concourse.mybir as mybir
    import concourse.tile as tile

    f32 = mybir.dt.float32
    f32r = mybir.dt.float32r
    f16 = mybir.dt.float16
    EXP = mybir.ActivationFunctionType.Exp
    RG = [list(range(P))]

    nc = bacc.Bacc("TRN2", target_bir_lowering=False, debug=False,
                   enable_asserts=False, num_devices=P)

    xs_h = nc.dram_tensor("xs_h", [NL, D], f16, kind="ExternalInput").ap()
    w_h = nc.dram_tensor("w_h", [WSH, D], f16, kind="ExternalInput").ap()
    outT = nc.dram_tensor("outT", [D, NL], f16, kind="ExternalOutput").ap()

    with tile.TileContext(nc) as tc:
        with (
            tc.tile_pool(name="const", bufs=1) as cpool,
            tc.tile_pool(name="proj", bufs=1) as ppool,
            tc.tile_pool(name="xts", bufs=4) as xtpool,
            tc.tile_pool(name="xns", bufs=4) as xnpool,
            tc.tile_pool(name="expt", bufs=8) as epool,
            tc.tile_pool(name="tail", bufs=1) as tpool,
            tc.tile_pool(name="dram", bufs=1, space="DRAM") as dpool,
            tc.tile_pool(name="ps_scores", bufs=2, space="PSUM") as ps_s,
            tc.tile_pool(name="ps_ut", bufs=1, space="PSUM") as ps_ut,
            tc.tile_pool(name="ps_sums", bufs=1, space="PSUM") as ps_sum,
        ):
            # ---- stage own shard + weights into DRAM, AllGather ----
            # natural-layout shard: fp16 -> SBUF -> f32r -> local DRAM -> AG
            xs_sb_h = cpool.tile([128, SB, D], f16, tag="xs_sb_h", name="xs_sb_h")
            nc.sync.dma_start(
                xs_sb_h[:], xs_h[:].rearrange("(a p) d -> p a d", p=128))
            xs_sb = cpool.tile([128, SB, D], f32r, tag="xs_sb", name="xs_sb")
            nc.vector.tensor_copy(xs_sb[:], xs_sb_h[:])
            xs_int = dpool.tile([NL, D], f32r, tag="xs_int", name="xs_int")
            nc.sync.dma_start(
                xs_int[:].rearrange("(a p) d -> p a d", p=128), xs_sb[:])
            xg = dpool.tile([N, D], f32r, tag="xg", name="xg",
                            addr_space="Shared")
            nc.gpsimd.collective_compute(
                "AllGather", mybir.AluOpType.bypass, replica_groups=RG,
                ins=[xs_int.opt()], outs=[xg.opt()])

            # transposed shard via XBAR dma-transpose: fp16 -> f32r -> AG
            xsT_sb = []
            xsT_int = dpool.tile([D, NL], f32r, tag="xsT_int", name="xsT_int")
            for h in range(2):
                trh = cpool.tile([128, NL], f16, tag=f"trh{h}", name=f"trh{h}")
                nc.sync.dma_start(
                    trh[:], xs_h[:, h * 128:(h + 1) * 128], transpose=True)
                trf = cpool.tile([128, NL], f32r, tag=f"trf{h}", name=f"trf{h}")
                nc.vector.tensor_copy(trf[:], trh[:])
                nc.sync.dma_start(xsT_int[h * 128:(h + 1) * 128, :], trf[:])
                xsT_sb.append(trf)
            xgT = dpool.tile([P * D, NL], f32r, tag="xgT", name="xgT",
                             addr_space="Shared")
            nc.gpsimd.collective_compute(
                "AllGather", mybir.AluOpType.bypass, replica_groups=RG,
                ins=[xsT_int.opt()], outs=[xgT.opt()])

            # packed weights [WQ; WK.T; WV]: shard -> AG -> SBUF f32r tiles
            w_sb_h = cpool.tile([WSH, D], f16, tag="w_sb_h", name="w_sb_h")
            nc.sync.dma_start(w_sb_h[:], w_h[:])
            w_int = dpool.tile([WSH, D], f16, tag="w_int", name="w_int")
            nc.sync.dma_start(w_int[:], w_sb_h[:])
            w_all = dpool.tile([3 * D, D], f16, tag="w_all", name="w_all",
                               addr_space="Shared")
            nc.gpsimd.collective_compute(
                "AllGather", mybir.AluOpType.bypass, replica_groups=RG,
                ins=[w_int.opt()], outs=[w_all.opt()])

            def wtiles(base, nm):
                out = []
                for h in range(2):
                    th = cpool.tile([128, D], f16, tag=f"{nm}h{h}",
                                    name=f"{nm}h{h}")
                    nc.sync.dma_start(
                        th[:], w_all[base + h * 128: base + (h + 1) * 128, :])
                    tf = cpool.tile([128, D], f32r, tag=f"{nm}{h}",
                                    name=f"{nm}{h}")
                    nc.vector.tensor_copy(tf[:], th[:])
                    out.append(tf)
                return out

            wq_t = wtiles(0, "wq")
            wkt_t = wtiles(D, "wkt")
            wv_t = wtiles(2 * D, "wv")

            # ---- constants ----
            ones_col = cpool.tile([128, 1], f32r, tag="ones_col", name="ones_col")
            ones_row = cpool.tile([1, 128], f32r, tag="ones_row", name="ones_row")
            ones_f32 = cpool.tile([128, 1], f32, tag="ones_f32", name="ones_f32")
            ones_f32r = cpool.tile([1, 128], f32, tag="ones_f32r", name="ones_f32r")
            bias_t = cpool.tile([128, 1], f32, tag="bias_t", name="bias_t")
            nc.vector.memset(ones_f32[:], 1.0)
            nc.vector.memset(ones_f32r[:], 1.0)
            nc.vector.tensor_copy(ones_col[:], ones_f32[:])
            nc.vector.tensor_copy(ones_row[:], ones_f32r[:])
            nc.vector.memset(bias_t[:], EXP_SHIFT)

            # ---- qT = WQ.T @ xT_local ; M = WK @ qT ----
            qT_t = [ppool.tile([128, NL], f32r, tag=f"qt{h}", name=f"qt{h}") for h in range(2)]
            m_t = [ppool.tile([128, NL], f32r, tag=f"m{h}", name=f"m{h}") for h in range(2)]
            for dst, lhs in ((qT_t, wq_t), (m_t, wkt_t)):
                src = xsT_sb if dst is qT_t else qT_t
                for mh in range(2):
                    for nh in range(2):
                        pp = ps_s.tile([128, 512], f32, tag="scores", name="scores")
                        for kp in range(2):
                            nc.tensor.matmul(
                                pp[:],
                                lhs[kp][:, mh * 128:(mh + 1) * 128],
                                src[kp][:, nh * 512:(nh + 1) * 512],
                                start=(kp == 0), stop=(kp == 1),
                            )
                        nc.vector.tensor_copy(
                            dst[mh][:, nh * 512:(nh + 1) * 512], pp[:])

            # ---- persistent accumulators ----
            ut_ps = [ps_ut.tile([128, NL], f32, tag=f"ut{h}", name=f"ut{h}") for h in range(2)]
            sums_ps = [ps_sum.tile([1, 512], f32, tag=f"sums{h}", name=f"sums{h}")
                       for h in range(2)]

            # ---- main k-loop ----
            for sb in range(N // (KC * SB)):
                xt_t = [xtpool.tile([128, KC * SB], f32r, tag=f"xt{h}", name=f"xt{h}")
                        for h in range(2)]
                for h in range(2):
                    nc.sync.dma_start(
                        xt_t[h][:],
                        xgT[sb * 2 * 128 + h * 128:sb * 2 * 128 + (h + 1) * 128,
                            :])
                xn_t = xnpool.tile([128, SB, D], f32r, tag="xn", name="xn")
                nc.sync.dma_start(
                    xn_t[:],
                    xg[sb * KC * SB:(sb + 1) * KC * SB, :]
                    .rearrange("(a p) d -> p a d", p=128))

                for j in range(SB):
                    c = sb * SB + j
                    first, last = (c == 0), (c == NCHUNK - 1)
                    exps = []
                    for qh in range(2):
                        sp = ps_s.tile([128, 512], f32, tag="scores", name="scores")
                        for kp in range(2):
                            nc.tensor.matmul(
                                sp[:],
                                xt_t[kp][:, j * KC:(j + 1) * KC],
                                m_t[kp][:, qh * 512:(qh + 1) * 512],
                                start=(kp == 0), stop=(kp == 1),
                            )
                        et = epool.tile([128, 512], f32r, tag="expt", name="expt")
                        nc.scalar.activation(et[:], sp[:], EXP, bias=bias_t[:])
                        exps.append(et)
                    for qh in range(2):
                        et = exps[qh]
                        nc.tensor.matmul(
                            sums_ps[qh][:], ones_col[:], et[:],
                            start=first, stop=last)
                        for dh in range(2):
                            nc.tensor.matmul(
                                ut_ps[dh][:, qh * 512:(qh + 1) * 512],
                                xn_t[:, j, dh * 128:(dh + 1) * 128],
                                et[:],
                                start=first, stop=last)

            # ---- tail: softmax normalize + WV projection ----
            sums_sb = tpool.tile([1, NL], f32, tag="sums_sb", name="sums_sb")
            for qh in range(2):
                nc.vector.tensor_copy(
                    sums_sb[:, qh * 512:(qh + 1) * 512], sums_ps[qh][:])
            recip_sb = tpool.tile([1, NL], f32r, tag="recip_sb", name="recip_sb")
            with nc.allow_low_precision(reason="f32r is 4-byte, same mantissa path"):
                nc.vector.reciprocal(recip_sb[:], sums_sb[:])

            rb_sb = tpool.tile([128, NL], f32, tag="rb_sb", name="rb_sb")
            for qh in range(2):
                rp = ps_s.tile([128, 512], f32, tag="scores", name="scores")
                nc.tensor.matmul(
                    rp[:], ones_row[:],
                    recip_sb[:, qh * 512:(qh + 1) * 512],
                    start=True, stop=True)
                nc.vector.tensor_copy(rb_sb[:, qh * 512:(qh + 1) * 512], rp[:])

            utn_sb = [tpool.tile([128, NL], f32r, tag=f"utn{h}", name=f"utn{h}")
                      for h in range(2)]
            for dh in range(2):
                nc.vector.tensor_mul(utn_sb[dh][:], ut_ps[dh][:], rb_sb[:])

            o_sb = [tpool.tile([128, NL], f16, tag=f"osb{h}", name=f"osb{h}") for h in range(2)]
            for mh in range(2):
                op = ps_ut.tile([128, NL], f32, tag=f"ut{mh}", name=f"ut{mh}")
                for nh in range(2):
                    for kp in range(2):
                        nc.tensor.matmul(
                            op[:, nh * 512:(nh + 1) * 512],
                            wv_t[kp][:, mh * 128:(mh + 1) * 128],
                            utn_sb[kp][:, nh * 512:(nh + 1) * 512],
                            start=(kp == 0), stop=(kp == 1),
                        )
                with nc.allow_low_precision(reason="fp16 output transport"):
                    nc.vector.tensor_copy(o_sb[mh][:], op[:])
                nc.sync.dma_start(outT[mh * 128:(mh + 1) * 128, :], o_sb[mh][:])

    nc.compile()
    return nc


def _get_nc():
    if "nc" not in _CACHE:
        _CACHE["nc"] = _build()
    return _CACHE["nc"]


def make_in_maps(input, WQ, WK, WV):
    """Per-core input maps: own fp16 x shard + 1/8 of packed [WQ; WK.T; WV]."""
    xh = np.ascontiguousarray(input, dtype=np.float32).astype(np.float16)
    wpack = np.concatenate(
        [np.asarray(WQ, dtype=np.float32),
         np.asarray(WK, dtype=np.float32).T,
         np.asarray(WV, dtype=np.float32)], axis=0).astype(np.float16)
    return [{
        "xs_h": np.ascontiguousarray(xh[c * NL:(c + 1) * NL]),
        "w_h": np.ascontiguousarray(wpack[c * WSH:(c + 1) * WSH]),
    } for c in range(P)]


def kernel(input, WQ, WK, WV):
    from concourse import bass_utils

    nc = _get_nc()
    in_maps = make_in_maps(input, WQ, WK, WV)
    res = bass_utils.run_bass_kernel_spmd(nc, in_maps, core_ids=list(range(P)))
    out = np.empty((N, D), dtype=np.float32)
    for c in range(P):
        out[c * NL:(c + 1) * NL, :] = res.results[c]["outT"].astype(np.float32).T
    return out


# revision 5
# speedup vs baseline: 10.6503x; 1.3721x over previous
"""Sequence-parallel self-attention kernel for 8 TRN2 NeuronCores.

Reference computation (N=8192, D=256, fp32):
    q = x @ WQ; k = x @ WK; v = x @ WV
    out = softmax(q @ k.T) @ v

Host->device traffic is the wall-clock bottleneck (axon tunnel ~35 MB/s), so
each core receives ONLY its own fp16 shard plus a 1/8 slice of the packed
weights (~0.55 MB/core instead of 17.8 MB/core replicated), and the full x is
reconstructed on-device with AllGathers over NeuronLink (~14 us each):

  per core c:
    xs_h  [1024, 256] fp16   own x rows (natural layout)
    w_h   [96, 256]   fp16   rows c*96..(c+1)*96 of packed [WQ; WK.T; WV]
  on device:
    AG#1: cast(xs_h)->f32r, gather -> xg  [8192, 256]   (natural x)
    AG#2: XBAR dma-transpose(xs_h)->f32r, gather -> xgT [2048, 1024]
          (8 stacked [256,1024] per-core transposed shards)
    AG#3: gather w_h -> w_all [768, 256] fp16, cast -> f32r weight tiles

Per-core algebra (identical to the proven replicated-input kernel; everything
stays transposed so softmax's k-reduction is a partition-axis ones-matmul):
    qT = WQ.T @ xT_local                      [256, 1024]
    M  = WK @ qT        (lhsT = WK.T)         [256, 1024]
    per k-chunk c (64 chunks of 128):
      scoresT = x_c @ M                       [128, 1024]   (= (q @ k.T).T chunk)
      expT    = exp(scoresT - 15)             (constant shift cancels in softmax)
      sums   += ones[128,1].T @ expT          [1, 1024]     (softmax denominator)
      UT     += x_c.T @ expT                  [256, 1024]   (= (attn_unnorm @ x).T)
    UTn  = UT * broadcast(1/sums)
    outT = WV.T @ UTn                         [256, 1024]   (fp16 out, host
                                                             transposes+upcasts)

All matmuls run as float32r (full PE rate at free-dim >= 256). fp16 transport
adds ~3e-3 rel err on top of f32r's ~1.5e-3 -- well under the 2e-2 gate.
"""

import numpy as np

N, D, P = 8192, 256, 8
NL = N // P          # 1024 q-rows per core
KC = 128             # k-chunk size (contraction tile)
NCHUNK = N // KC     # 64
SB = 8               # k-chunks per DMA superblock
WSH = 3 * D // P     # 96 packed-weight rows per core
EXP_SHIFT = -15.0    # exp(s - 15): keeps ACT exp-table args in a good range

_CACHE = {}


def _build():
    import concourse.bacc as bacc
    import concourse.mybir as mybir
    import concourse.tile as tile

    f32 = mybir.dt.float32
    f32r = mybir.dt.float32r
    f16 = mybir.dt.float16
    EXP = mybir.ActivationFunctionType.Exp
    RG = [list(range(P))]

    nc = bacc.Bacc("TRN2", target_bir_lowering=False, debug=False,
                   enable_asserts=False, num_devices=P)

    xs_h = nc.dram_tensor("xs_h", [NL, D], f16, kind="ExternalInput").ap()
    w_h = nc.dram_tensor("w_h", [WSH, D], f16, kind="ExternalInput").ap()
    outT = nc.dram_tensor("outT", [D, NL], f16, kind="ExternalOutput").ap()

    with tile.TileContext(nc) as tc:
        with (
            tc.tile_pool(name="const", bufs=1) as cpool,
            tc.tile_pool(name="proj", bufs=1) as ppool,
            tc.tile_pool(name="xts", bufs=4) as xtpool,
            tc.tile_pool(name="xns", bufs=4) as xnpool,
            tc.tile_pool(name="expt", bufs=8) as epool,
            tc.tile_pool(name="tail", bufs=1) as tpool,
            tc.tile_pool(name="dram", bufs=1, space="DRAM") as dpool,
            tc.tile_pool(name="ps_scores", bufs=2, space="PSUM") as ps_s,
            tc.tile_pool(name="ps_ut", bufs=1, space="PSUM") as ps_ut,
            tc.tile_pool(name="ps_sums", bufs=1, space="PSUM") as ps_sum,
        ):
            # ---- stage own shard + weights into DRAM, AllGather ----
            # natural-layout shard: fp16 -> SBUF -> f32r -> local DRAM -> AG
            xs_sb_h = cpool.tile([128, SB, D], f16, tag="xs_sb_h", name="xs_sb_h")
            nc.sync.dma_start(
                xs_sb_h[:], xs_h[:].rearrange("(a p) d -> p a d", p=128))
            xs_sb = cpool.tile([128, SB, D], f32r, tag="xs_sb", name="xs_sb")
            nc.vector.tensor_copy(xs_sb[:], xs_sb_h[:])
            xs_int = dpool.tile([NL, D], f32r, tag="xs_int", name="xs_int")
            nc.sync.dma_start(
                xs_int[:].rearrange("(a p) d -> p a d", p=128), xs_sb[:])
            xg = dpool.tile([N, D], f32r, tag="xg", name="xg",
                            addr_space="Shared")
            nc.gpsimd.collective_compute(
                "AllGather", mybir.AluOpType.bypass, replica_groups=RG,
                ins=[xs_int.opt()], outs=[xg.opt()])

            # transposed shard via XBAR dma-transpose: fp16 -> f32r -> AG
            xsT_sb = []
            xsT_int = dpool.tile([D, NL], f32r, tag="xsT_int", name="xsT_int")
            for h in range(2):
                trh = cpool.tile([128, NL], f16, tag=f"trh{h}", name=f"trh{h}")
                nc.sync.dma_start(
                    trh[:], xs_h[:, h * 128:(h + 1) * 128], transpose=True)
                trf = cpool.tile([128, NL], f32r, tag=f"trf{h}", name=f"trf{h}")
                nc.vector.tensor_copy(trf[:], trh[:])
                nc.sync.dma_start(xsT_int[h * 128:(h + 1) * 128, :], trf[:])
                xsT_sb.append(trf)
            xgT = dpool.tile([P * D, NL], f32r, tag="xgT", name="xgT",
                             addr_space="Shared")
            nc.gpsimd.collective_compute(
                "AllGather", mybir.AluOpType.bypass, replica_groups=RG,
                ins=[xsT_int.opt()], outs=[xgT.opt()])

            # packed weights [WQ; WK.T; WV]: shard -> AG -> SBUF f32r tiles
            w_sb_h = cpool.tile([WSH, D], f16, tag="w_sb_h", name="w_sb_h")
            nc.sync.dma_start(w_sb_h[:], w_h[:])
            w_int = dpool.tile([WSH, D], f16, tag="w_int", name="w_int")
            nc.sync.dma_start(w_int[:], w_sb_h[:])
            w_all = dpool.tile([3 * D, D], f16, tag="w_all", name="w_all",
                               addr_space="Shared")
            nc.gpsimd.collective_compute(
                "AllGather", mybir.AluOpType.bypass, replica_groups=RG,
                ins=[w_int.opt()], outs=[w_all.opt()])

            def wtiles(base, nm):
                out = []
                for h in range(2):
                    th = cpool.tile([128, D], f16, tag=f"{nm}h{h}",
                                    name=f"{nm}h{h}")
                    nc.sync.dma_start(
                        th[:], w_all[base + h * 128: base + (h + 1) * 128, :])
                    tf = cpool.tile([128, D], f32r, tag=f"{nm}{h}",
                                    name=f"{nm}{h}")
                    nc.vector.tensor_copy(tf[:], th[:])
                    out.append(tf)
                return out

            wq_t = wtiles(0, "wq")
            wkt_t = wtiles(D, "wkt")
            wv_t = wtiles(2 * D, "wv")

            # ---- constants ----
            ones_col = cpool.tile([128, 1], f32r, tag="ones_col", name="ones_col")
            ones_row = cpool.tile([1, 128], f32r, tag="ones_row", name="ones_row")
            ones_f32 = cpool.tile([128, 1], f32, tag="ones_f32", name="ones_f32")
            ones_f32r = cpool.tile([1, 128], f32, tag="ones_f32r", name="ones_f32r")
            bias_t = cpool.tile([128, 1], f32, tag="bias_t", name="bias_t")
            nc.vector.memset(ones_f32[:], 1.0)
            nc.vector.memset(ones_f32r[:], 1.0)
            nc.vector.tensor_copy(ones_col[:], ones_f32[:])
            nc.vector.tensor_copy(ones_row[:], ones_f32r[:])
            nc.vector.memset(bias_t[:], EXP_SHIFT)

            # ---- qT = WQ.T @ xT_local ; M = WK @ qT ----
            qT_t = [ppool.tile([128, NL], f32r, tag=f"qt{h}", name=f"qt{h}") for h in range(2)]
            m_t = [ppool.tile([128, NL], f32r, tag=f"m{h}", name=f"m{h}") for h in range(2)]
            for dst, lhs in ((qT_t, wq_t), (m_t, wkt_t)):
                src = xsT_sb if dst is qT_t else qT_t
                for mh in range(2):
                    for nh in range(2):
                        pp = ps_s.tile([128, 512], f32, tag="scores", name="scores")
                        for kp in range(2):
                            nc.tensor.matmul(
                                pp[:],
                                lhs[kp][:, mh * 128:(mh + 1) * 128],
                                src[kp][:, nh * 512:(nh + 1) * 512],
                                start=(kp == 0), stop=(kp == 1),
                            )
                        nc.vector.tensor_copy(
                            dst[mh][:, nh * 512:(nh + 1) * 512], pp[:])

            # ---- persistent accumulators ----
            ut_ps = [ps_ut.tile([128, NL], f32, tag=f"ut{h}", name=f"ut{h}") for h in range(2)]
            sums_ps = [ps_sum.tile([1, 512], f32, tag=f"sums{h}", name=f"sums{h}")
                       for h in range(2)]

            # ---- main k-loop ----
            for sb in range(N // (KC * SB)):
                xt_t = [xtpool.tile([128, KC * SB], f32r, tag=f"xt{h}", name=f"xt{h}")
                        for h in range(2)]
                for h in range(2):
                    nc.sync.dma_start(
                        xt_t[h][:],
                        xgT[sb * 2 * 128 + h * 128:sb * 2 * 128 + (h + 1) * 128,
                            :])
                xn_t = xnpool.tile([128, SB, D], f32r, tag="xn", name="xn")
                nc.sync.dma_start(
                    xn_t[:],
                    xg[sb * KC * SB:(sb + 1) * KC * SB, :]
                    .rearrange("(a p) d -> p a d", p=128))

                for j in range(SB):
                    c = sb * SB + j
                    first, last = (c == 0), (c == NCHUNK - 1)
                    exps = []
                    for qh in range(2):
                        sp = ps_s.tile([128, 512], f32, tag="scores", name="scores")
                        for kp in range(2):
                            nc.tensor.matmul(
                                sp[:],
                                xt_t[kp][:, j * KC:(j + 1) * KC],
                                m_t[kp][:, qh * 512:(qh + 1) * 512],
                                start=(kp == 0), stop=(kp == 1),
                            )
                        et = epool.tile([128, 512], f32r, tag="expt", name="expt")
                        nc.scalar.activation(et[:], sp[:], EXP, bias=bias_t[:])
                        exps.append(et)
                    for qh in range(2):
                        et = exps[qh]
                        nc.tensor.matmul(
                            sums_ps[qh][:], ones_col[:], et[:],
                            start=first, stop=last)
                        for dh in range(2):
                            nc.tensor.matmul(
                                ut_ps[dh][:, qh * 512:(qh + 1) * 512],
                                xn_t[:, j, dh * 128:(dh + 1) * 128],
                                et[:],
                                start=first, stop=last)

            # ---- tail: softmax normalize + WV projection ----
            sums_sb = tpool.tile([1, NL], f32, tag="sums_sb", name="sums_sb")
            for qh in range(2):
                nc.vector.tensor_copy(
                    sums_sb[:, qh * 512:(qh + 1) * 512], sums_ps[qh][:])
            recip_sb = tpool.tile([1, NL], f32r, tag="recip_sb", name="recip_sb")
            with nc.allow_low_precision(reason="f32r is 4-byte, same mantissa path"):
                nc.vector.reciprocal(recip_sb[:], sums_sb[:])

            rb_sb = tpool.tile([128, NL], f32, tag="rb_sb", name="rb_sb")
            for qh in range(2):
                rp = ps_s.tile([128, 512], f32, tag="scores", name="scores")
                nc.tensor.matmul(
                    rp[:], ones_row[:],
                    recip_sb[:, qh * 512:(qh + 1) * 512],
                    start=True, stop=True)
                nc.vector.tensor_copy(rb_sb[:, qh * 512:(qh + 1) * 512], rp[:])

            utn_sb = [tpool.tile([128, NL], f32r, tag=f"utn{h}", name=f"utn{h}")
                      for h in range(2)]
            for dh in range(2):
                nc.vector.tensor_mul(utn_sb[dh][:], ut_ps[dh][:], rb_sb[:])

            o_sb = [tpool.tile([128, NL], f16, tag=f"osb{h}", name=f"osb{h}") for h in range(2)]
            for mh in range(2):
                op = ps_ut.tile([128, NL], f32, tag=f"ut{mh}", name=f"ut{mh}")
                for nh in range(2):
                    for kp in range(2):
                        nc.tensor.matmul(
                            op[:, nh * 512:(nh + 1) * 512],
                            wv_t[kp][:, mh * 128:(mh + 1) * 128],
                            utn_sb[kp][:, nh * 512:(nh + 1) * 512],
                            start=(kp == 0), stop=(kp == 1),
                        )
                with nc.allow_low_precision(reason="fp16 output transport"):
                    nc.vector.tensor_copy(o_sb[mh][:], op[:])
                nc.sync.dma_start(outT[mh * 128:(mh + 1) * 128, :], o_sb[mh][:])

    nc.compile()
    return nc


def _setup_jax_cache():
    """Persistent XLA compilation cache: run_bass_kernel_spmd re-jits a fresh
    closure every call, so without this each call pays ~100ms of XLA
    recompile for the identical HLO."""
    if "jaxcache" in _CACHE:
        return
    import jax

    jax.config.update("jax_compilation_cache_dir", "/tmp/jaxcache")
    jax.config.update("jax_persistent_cache_min_entry_size_bytes", 0)
    jax.config.update("jax_persistent_cache_min_compile_time_secs", 0)
    _CACHE["jaxcache"] = True


def _get_nc():
    if "nc" not in _CACHE:
        _setup_jax_cache()
        _CACHE["nc"] = _build()
    return _CACHE["nc"]


def make_in_maps(input, WQ, WK, WV):
    """Per-core input maps: own fp16 x shard + 1/8 of packed [WQ; WK.T; WV]."""
    xh = np.ascontiguousarray(input, dtype=np.float32).astype(np.float16)
    wpack = np.concatenate(
        [np.asarray(WQ, dtype=np.float32),
         np.asarray(WK, dtype=np.float32).T,
         np.asarray(WV, dtype=np.float32)], axis=0).astype(np.float16)
    return [{
        "xs_h": np.ascontiguousarray(xh[c * NL:(c + 1) * NL]),
        "w_h": np.ascontiguousarray(wpack[c * WSH:(c + 1) * WSH]),
    } for c in range(P)]


def kernel(input, WQ, WK, WV):
    from concourse import bass_utils

    nc = _get_nc()
    in_maps = make_in_maps(input, WQ, WK, WV)
    res = bass_utils.run_bass_kernel_spmd(nc, in_maps, core_ids=list(range(P)))
    out = np.empty((N, D), dtype=np.float32)
    for c in range(P):
        out[c * NL:(c + 1) * NL, :] = res.results[c]["outT"].astype(np.float32).T
    return out


# revision 18
# speedup vs baseline: 11.4934x; 1.0792x over previous
"""Sequence-parallel self-attention kernel for 8 TRN2 NeuronCores.

Reference computation (N=8192, D=256, fp32):
    q = x @ WQ; k = x @ WK; v = x @ WV
    out = softmax(q @ k.T) @ v

Host->device traffic is the wall-clock bottleneck (axon tunnel ~35 MB/s), so
each core receives ONLY its own fp16 shard plus a 1/8 slice of the packed
weights (~0.55 MB/core instead of 17.8 MB/core replicated), and the full x is
reconstructed on-device with AllGathers over NeuronLink (~14 us each):

  per core c:
    xs_h  [1024, 256] fp16   own x rows (natural layout)
    w_h   [96, 256]   fp16   rows c*96..(c+1)*96 of packed [WQ; WK.T; WV]
  on device:
    AG#1: cast(xs_h)->f32r, gather -> xg  [8192, 256]   (natural x)
    AG#2: XBAR dma-transpose(xs_h)->f32r, gather -> xgT [2048, 1024]
          (8 stacked [256,1024] per-core transposed shards)
    AG#3: gather w_h -> w_all [768, 256] fp16, cast -> f32r weight tiles

Per-core algebra (identical to the proven replicated-input kernel; everything
stays transposed so softmax's k-reduction is a partition-axis ones-matmul):
    qT = WQ.T @ xT_local                      [256, 1024]
    M  = WK @ qT        (lhsT = WK.T)         [256, 1024]
    per k-chunk c (64 chunks of 128):
      scoresT = x_c @ M                       [128, 1024]   (= (q @ k.T).T chunk)
      expT    = exp(scoresT - 15)             (constant shift cancels in softmax)
      sums   += ones[128,1].T @ expT          [1, 1024]     (softmax denominator)
      UT     += x_c.T @ expT                  [256, 1024]   (= (attn_unnorm @ x).T)
    UTn  = UT * broadcast(1/sums)
    outT = WV.T @ UTn                         [256, 1024]   (int8 + f32 absmax
                                                             scale; host
                                                             dequantizes + .T)

All matmuls run as float32r (full PE rate at free-dim >= 256). fp16 transport
adds ~3e-3 rel err on top of f32r's ~1.5e-3 -- well under the 2e-2 gate.
"""

import numpy as np

N, D, P = 8192, 256, 8
NL = N // P          # 1024 q-rows per core
KC = 128             # k-chunk size (contraction tile)
NCHUNK = N // KC     # 64
SB = 8               # k-chunks per DMA superblock
WSH = 3 * D // P     # 96 packed-weight rows per core
EXP_SHIFT = -15.0    # exp(s - 15): keeps ACT exp-table args in a good range
QDEN = 120.0         # int8 quant denominator; headroom vs 127 absorbs the
                     # ~1% error of the DVE reciprocal so +max never wraps

_CACHE = {}


def _build():
    import concourse.bacc as bacc
    import concourse.mybir as mybir
    import concourse.tile as tile

    import concourse.bass_isa as bass_isa

    f32 = mybir.dt.float32
    f32r = mybir.dt.float32r
    f16 = mybir.dt.float16
    i8 = mybir.dt.int8
    EXP = mybir.ActivationFunctionType.Exp
    RCP = mybir.ActivationFunctionType.Reciprocal
    COPY = mybir.ActivationFunctionType.Copy
    RG = [list(range(P))]

    nc = bacc.Bacc("TRN2", target_bir_lowering=False, debug=False,
                   enable_asserts=False, num_devices=P)

    xs_h = nc.dram_tensor("xs_h", [NL, D], f16, kind="ExternalInput").ap()
    w_h = nc.dram_tensor("w_h", [WSH, D], f16, kind="ExternalInput").ap()
    outT = nc.dram_tensor("outT", [D, NL], i8, kind="ExternalOutput").ap()
    om = nc.dram_tensor("om", [1, 1], f32, kind="ExternalOutput").ap()

    with tile.TileContext(nc) as tc:
        with (
            tc.tile_pool(name="const", bufs=1) as cpool,
            tc.tile_pool(name="proj", bufs=1) as ppool,
            tc.tile_pool(name="xts", bufs=4) as xtpool,
            tc.tile_pool(name="xns", bufs=4) as xnpool,
            tc.tile_pool(name="expt", bufs=8) as epool,
            tc.tile_pool(name="tail", bufs=1) as tpool,
            tc.tile_pool(name="dram", bufs=1, space="DRAM") as dpool,
            tc.tile_pool(name="ps_scores", bufs=2, space="PSUM") as ps_s,
            tc.tile_pool(name="ps_ut", bufs=1, space="PSUM") as ps_ut,
            tc.tile_pool(name="ps_sums", bufs=1, space="PSUM") as ps_sum,
        ):
            # ---- stage own shard + weights into DRAM, AllGather ----
            # natural-layout shard: fp16 -> SBUF -> f32r -> local DRAM -> AG
            xs_sb_h = cpool.tile([128, SB, D], f16, tag="xs_sb_h", name="xs_sb_h")
            nc.sync.dma_start(
                xs_sb_h[:], xs_h[:].rearrange("(a p) d -> p a d", p=128))
            xs_sb = cpool.tile([128, SB, D], f32r, tag="xs_sb", name="xs_sb")
            nc.vector.tensor_copy(xs_sb[:], xs_sb_h[:])
            xs_int = dpool.tile([NL, D], f32r, tag="xs_int", name="xs_int")
            nc.sync.dma_start(
                xs_int[:].rearrange("(a p) d -> p a d", p=128), xs_sb[:])
            xg = dpool.tile([N, D], f32r, tag="xg", name="xg",
                            addr_space="Shared")
            nc.gpsimd.collective_compute(
                "AllGather", mybir.AluOpType.bypass, replica_groups=RG,
                ins=[xs_int.opt()], outs=[xg.opt()])

            # transposed shard via XBAR dma-transpose: fp16 -> f32r -> AG
            xsT_sb = []
            xsT_int = dpool.tile([D, NL], f32r, tag="xsT_int", name="xsT_int")
            for h in range(2):
                trh = cpool.tile([128, NL], f16, tag=f"trh{h}", name=f"trh{h}")
                nc.sync.dma_start(
                    trh[:], xs_h[:, h * 128:(h + 1) * 128], transpose=True)
                trf = cpool.tile([128, NL], f32r, tag=f"trf{h}", name=f"trf{h}")
                nc.vector.tensor_copy(trf[:], trh[:])
                nc.sync.dma_start(xsT_int[h * 128:(h + 1) * 128, :], trf[:])
                xsT_sb.append(trf)
            xgT = dpool.tile([P * D, NL], f32r, tag="xgT", name="xgT",
                             addr_space="Shared")
            nc.gpsimd.collective_compute(
                "AllGather", mybir.AluOpType.bypass, replica_groups=RG,
                ins=[xsT_int.opt()], outs=[xgT.opt()])

            # packed weights [WQ; WK.T; WV]: shard -> AG -> SBUF f32r tiles
            w_sb_h = cpool.tile([WSH, D], f16, tag="w_sb_h", name="w_sb_h")
            nc.sync.dma_start(w_sb_h[:], w_h[:])
            w_int = dpool.tile([WSH, D], f16, tag="w_int", name="w_int")
            nc.sync.dma_start(w_int[:], w_sb_h[:])
            w_all = dpool.tile([3 * D, D], f16, tag="w_all", name="w_all",
                               addr_space="Shared")
            nc.gpsimd.collective_compute(
                "AllGather", mybir.AluOpType.bypass, replica_groups=RG,
                ins=[w_int.opt()], outs=[w_all.opt()])

            def wtiles(base, nm):
                out = []
                for h in range(2):
                    th = cpool.tile([128, D], f16, tag=f"{nm}h{h}",
                                    name=f"{nm}h{h}")
                    nc.sync.dma_start(
                        th[:], w_all[base + h * 128: base + (h + 1) * 128, :])
                    tf = cpool.tile([128, D], f32r, tag=f"{nm}{h}",
                                    name=f"{nm}{h}")
                    nc.vector.tensor_copy(tf[:], th[:])
                    out.append(tf)
                return out

            wq_t = wtiles(0, "wq")
            wkt_t = wtiles(D, "wkt")
            wv_t = wtiles(2 * D, "wv")

            # ---- constants ----
            ones_col = cpool.tile([128, 1], f32r, tag="ones_col", name="ones_col")
            ones_row = cpool.tile([1, 128], f32r, tag="ones_row", name="ones_row")
            ones_f32 = cpool.tile([128, 1], f32, tag="ones_f32", name="ones_f32")
            ones_f32r = cpool.tile([1, 128], f32, tag="ones_f32r", name="ones_f32r")
            bias_t = cpool.tile([128, 1], f32, tag="bias_t", name="bias_t")
            nc.vector.memset(ones_f32[:], 1.0)
            nc.vector.memset(ones_f32r[:], 1.0)
            nc.vector.tensor_copy(ones_col[:], ones_f32[:])
            nc.vector.tensor_copy(ones_row[:], ones_f32r[:])
            nc.vector.memset(bias_t[:], EXP_SHIFT)

            # ---- qT = WQ.T @ xT_local ; M = WK @ qT ----
            qT_t = [ppool.tile([128, NL], f32r, tag=f"qt{h}", name=f"qt{h}") for h in range(2)]
            m_t = [ppool.tile([128, NL], f32r, tag=f"m{h}", name=f"m{h}") for h in range(2)]
            for dst, lhs in ((qT_t, wq_t), (m_t, wkt_t)):
                src = xsT_sb if dst is qT_t else qT_t
                for mh in range(2):
                    for nh in range(2):
                        pp = ps_s.tile([128, 512], f32, tag="scores", name="scores")
                        for kp in range(2):
                            nc.tensor.matmul(
                                pp[:],
                                lhs[kp][:, mh * 128:(mh + 1) * 128],
                                src[kp][:, nh * 512:(nh + 1) * 512],
                                start=(kp == 0), stop=(kp == 1),
                            )
                        nc.vector.tensor_copy(
                            dst[mh][:, nh * 512:(nh + 1) * 512], pp[:])

            # ---- persistent accumulators ----
            ut_ps = [ps_ut.tile([128, NL], f32, tag=f"ut{h}", name=f"ut{h}") for h in range(2)]
            sums_ps = [ps_sum.tile([1, 512], f32, tag=f"sums{h}", name=f"sums{h}")
                       for h in range(2)]

            # ---- main k-loop ----
            for sb in range(N // (KC * SB)):
                xt_t = [xtpool.tile([128, KC * SB], f32r, tag=f"xt{h}", name=f"xt{h}")
                        for h in range(2)]
                for h in range(2):
                    nc.sync.dma_start(
                        xt_t[h][:],
                        xgT[sb * 2 * 128 + h * 128:sb * 2 * 128 + (h + 1) * 128,
                            :])
                xn_t = xnpool.tile([128, SB, D], f32r, tag="xn", name="xn")
                nc.sync.dma_start(
                    xn_t[:],
                    xg[sb * KC * SB:(sb + 1) * KC * SB, :]
                    .rearrange("(a p) d -> p a d", p=128))

                for j in range(SB):
                    c = sb * SB + j
                    first, last = (c == 0), (c == NCHUNK - 1)
                    exps = []
                    for qh in range(2):
                        sp = ps_s.tile([128, 512], f32, tag="scores", name="scores")
                        for kp in range(2):
                            nc.tensor.matmul(
                                sp[:],
                                xt_t[kp][:, j * KC:(j + 1) * KC],
                                m_t[kp][:, qh * 512:(qh + 1) * 512],
                                start=(kp == 0), stop=(kp == 1),
                            )
                        et = epool.tile([128, 512], f32r, tag="expt", name="expt")
                        nc.scalar.activation(et[:], sp[:], EXP, bias=bias_t[:])
                        exps.append(et)
                    for qh in range(2):
                        et = exps[qh]
                        nc.tensor.matmul(
                            sums_ps[qh][:], ones_col[:], et[:],
                            start=first, stop=last)
                        for dh in range(2):
                            nc.tensor.matmul(
                                ut_ps[dh][:, qh * 512:(qh + 1) * 512],
                                xn_t[:, j, dh * 128:(dh + 1) * 128],
                                et[:],
                                start=first, stop=last)

            # ---- tail: softmax normalize + WV projection ----
            sums_sb = tpool.tile([1, NL], f32, tag="sums_sb", name="sums_sb")
            for qh in range(2):
                nc.vector.tensor_copy(
                    sums_sb[:, qh * 512:(qh + 1) * 512], sums_ps[qh][:])
            recip_sb = tpool.tile([1, NL], f32r, tag="recip_sb", name="recip_sb")
            with nc.allow_low_precision(reason="f32r is 4-byte, same mantissa path"):
                nc.vector.reciprocal(recip_sb[:], sums_sb[:])

            rb_sb = tpool.tile([128, NL], f32, tag="rb_sb", name="rb_sb")
            for qh in range(2):
                rp = ps_s.tile([128, 512], f32, tag="scores", name="scores")
                nc.tensor.matmul(
                    rp[:], ones_row[:],
                    recip_sb[:, qh * 512:(qh + 1) * 512],
                    start=True, stop=True)
                nc.vector.tensor_copy(rb_sb[:, qh * 512:(qh + 1) * 512], rp[:])

            utn_sb = [tpool.tile([128, NL], f32r, tag=f"utn{h}", name=f"utn{h}")
                      for h in range(2)]
            for dh in range(2):
                nc.vector.tensor_mul(utn_sb[dh][:], ut_ps[dh][:], rb_sb[:])

            # WV projection into two live PSUM tiles, then int8-quantize with a
            # per-core absmax scale (int8 + f32 scale halves the output bytes;
            # quantization error ~m/240 is ~4e-3 of the rel-err denominator)
            o_f32 = []
            am = tpool.tile([128, 2], f32, tag="am", name="am")
            for mh in range(2):
                op = ps_ut.tile([128, NL], f32, tag=f"ut{mh}", name=f"ut{mh}")
                for nh in range(2):
                    for kp in range(2):
                        nc.tensor.matmul(
                            op[:, nh * 512:(nh + 1) * 512],
                            wv_t[kp][:, mh * 128:(mh + 1) * 128],
                            utn_sb[kp][:, nh * 512:(nh + 1) * 512],
                            start=(kp == 0), stop=(kp == 1),
                        )
                of = tpool.tile([128, NL], f32, tag=f"of{mh}", name=f"of{mh}")
                nc.vector.tensor_copy(of[:], op[:])
                nc.vector.reduce_max(
                    am[:, mh:mh + 1], of[:], axis=mybir.AxisListType.X,
                    apply_absolute_value=True)
                o_f32.append(of)
            amax = tpool.tile([128, 1], f32, tag="amax", name="amax")
            nc.vector.reduce_max(amax[:], am[:], axis=mybir.AxisListType.X)
            nc.gpsimd.partition_all_reduce(
                amax[:], amax[:], channels=128,
                reduce_op=bass_isa.ReduceOp.absmax)
            sc126 = tpool.tile([128, 1], f32, tag="sc126", name="sc126")
            nc.scalar.activation(sc126[:], amax[:], COPY, scale=1.0 / QDEN)
            rcp = tpool.tile([128, 1], f32, tag="rcp", name="rcp")
            nc.vector.reciprocal(rcp[:], sc126[:])
            # NOTE: the int8 payload is DMA'd under an int32 bitcast. A plain
            # int8 SBUF->DRAM DMA corrupts the data on this hardware (each
            # 32-bit word gets an fp32-mantissa-style rounding: +0x800 then
            # low 12 bits cleared); 4-byte elements take the normal path.
            i32 = mybir.dt.int32
            o_sb = [tpool.tile([128, NL], i8, tag=f"osb{h}", name=f"osb{h}") for h in range(2)]
            for mh in range(2):
                with nc.allow_low_precision(reason="int8 output transport"):
                    nc.vector.tensor_scalar_mul(o_sb[mh][:], o_f32[mh][:],
                                                rcp[:])
                # gpsimd queue, NOT sync: late sync-queue stores corrupt the
                # payload in this build (32-bit words get an fp32-style
                # low-12-bit rounding); the gpsimd DGE ring is clean.
                nc.gpsimd.dma_start(
                    outT[mh * 128:(mh + 1) * 128, :], o_sb[mh][:])
            nc.gpsimd.dma_start(om[:], amax[0:1, 0:1])

    nc.compile()
    return nc


def _setup_jax_cache():
    """Persistent XLA compilation cache: run_bass_kernel_spmd re-jits a fresh
    closure every call, so without this each call pays ~100ms of XLA
    recompile for the identical HLO."""
    if "jaxcache" in _CACHE:
        return
    import jax

    jax.config.update("jax_compilation_cache_dir", "/tmp/jaxcache")
    jax.config.update("jax_persistent_cache_min_entry_size_bytes", 0)
    jax.config.update("jax_persistent_cache_min_compile_time_secs", 0)
    _CACHE["jaxcache"] = True


def _get_nc():
    if "nc" not in _CACHE:
        _setup_jax_cache()
        _CACHE["nc"] = _build()
    return _CACHE["nc"]


def make_in_maps(input, WQ, WK, WV):
    """Per-core input maps: own fp16 x shard + 1/8 of packed [WQ; WK.T; WV]."""
    xh = np.ascontiguousarray(input, dtype=np.float32).astype(np.float16)
    wpack = np.concatenate(
        [np.asarray(WQ, dtype=np.float32),
         np.asarray(WK, dtype=np.float32).T,
         np.asarray(WV, dtype=np.float32)], axis=0).astype(np.float16)
    return [{
        "xs_h": np.ascontiguousarray(xh[c * NL:(c + 1) * NL]),
        "w_h": np.ascontiguousarray(wpack[c * WSH:(c + 1) * WSH]),
    } for c in range(P)]


def kernel(input, WQ, WK, WV):
    from concourse import bass_utils

    nc = _get_nc()
    in_maps = make_in_maps(input, WQ, WK, WV)
    res = bass_utils.run_bass_kernel_spmd(nc, in_maps, core_ids=list(range(P)))
    out = np.empty((N, D), dtype=np.float32)
    for c in range(P):
        scale = float(res.results[c]["om"][0, 0]) / QDEN
        out[c * NL:(c + 1) * NL, :] = (
            res.results[c]["outT"].astype(np.float32) * scale).T
    return out


# revision 20
# speedup vs baseline: 11.8938x; 1.0348x over previous
"""Sequence-parallel self-attention kernel for 8 TRN2 NeuronCores.

Reference computation (N=8192, D=256, fp32):
    q = x @ WQ; k = x @ WK; v = x @ WV
    out = softmax(q @ k.T) @ v

Host->device traffic is the wall-clock bottleneck (axon tunnel ~35 MB/s), so
each core receives ONLY its own fp16 shard plus a 1/8 slice of the packed
weights (~0.55 MB/core instead of 17.8 MB/core replicated), and the full x is
reconstructed on-device with AllGathers over NeuronLink (~14 us each):

  per core c:
    xs_h  [1024, 256] fp16   own x rows (natural layout)
    w_h   [96, 256]   fp16   rows c*96..(c+1)*96 of packed [WQ; WK.T; WV]
  on device:
    AG#1: cast(xs_h)->f32r, gather -> xg  [8192, 256]   (natural x)
    AG#2: XBAR dma-transpose(xs_h)->f32r, gather -> xgT [2048, 1024]
          (8 stacked [256,1024] per-core transposed shards)
    AG#3: gather w_h -> w_all [768, 256] fp16, cast -> f32r weight tiles

Per-core algebra (identical to the proven replicated-input kernel; everything
stays transposed so softmax's k-reduction is a partition-axis ones-matmul):
    qT = WQ.T @ xT_local                      [256, 1024]
    M  = WK @ qT        (lhsT = WK.T)         [256, 1024]
    per k-chunk c (64 chunks of 128):
      scoresT = x_c @ M                       [128, 1024]   (= (q @ k.T).T chunk)
      expT    = exp(scoresT - 15)             (constant shift cancels in softmax)
      sums   += ones[128,1].T @ expT          [1, 1024]     (softmax denominator)
      UT     += x_c.T @ expT                  [256, 1024]   (= (attn_unnorm @ x).T)
    UTn  = UT * broadcast(1/sums)
    outT = WV.T @ UTn                         [256, 1024]   (int8 + f32 absmax
                                                             scale; host
                                                             dequantizes + .T)

All matmuls run as float32r (full PE rate at free-dim >= 256). fp16 input
transport + f32r compute + int8 output quantization land at ~5.7e-3 rel err
-- 3.5x under the 2e-2 gate.

Hardware quirk found while tuning: late SBUF->DRAM stores issued on the sync
DMA queue corrupt their payload in this build (every 32-bit word of some 4KB
spans gets +0x800 added then its low 12 bits cleared -- an fp32-mantissa-style
rounding). The output stores therefore go through the gpsimd DMA queue, which
is unaffected.
"""

import numpy as np

N, D, P = 8192, 256, 8
NL = N // P          # 1024 q-rows per core
KC = 128             # k-chunk size (contraction tile)
NCHUNK = N // KC     # 64
SB = 8               # k-chunks per DMA superblock
WSH = 3 * D // P     # 96 packed-weight rows per core
EXP_SHIFT = -15.0    # exp(s - 15): keeps ACT exp-table args in a good range
QDEN = 120.0         # int8 quant denominator; headroom vs 127 absorbs the
                     # ~1% error of the DVE reciprocal so +max never wraps

_CACHE = {}


def _build():
    import concourse.bacc as bacc
    import concourse.mybir as mybir
    import concourse.tile as tile

    import concourse.bass_isa as bass_isa

    f32 = mybir.dt.float32
    f32r = mybir.dt.float32r
    f16 = mybir.dt.float16
    i8 = mybir.dt.int8
    EXP = mybir.ActivationFunctionType.Exp
    COPY = mybir.ActivationFunctionType.Copy
    RG = [list(range(P))]

    nc = bacc.Bacc("TRN2", target_bir_lowering=False, debug=False,
                   enable_asserts=False, num_devices=P)

    xs_h = nc.dram_tensor("xs_h", [NL, D], f16, kind="ExternalInput").ap()
    w_h = nc.dram_tensor("w_h", [WSH, D], f16, kind="ExternalInput").ap()
    outT = nc.dram_tensor("outT", [D, NL], i8, kind="ExternalOutput").ap()
    om = nc.dram_tensor("om", [1, 1], f32, kind="ExternalOutput").ap()

    with tile.TileContext(nc) as tc:
        with (
            tc.tile_pool(name="const", bufs=1) as cpool,
            tc.tile_pool(name="proj", bufs=1) as ppool,
            tc.tile_pool(name="xts", bufs=4) as xtpool,
            tc.tile_pool(name="xns", bufs=4) as xnpool,
            tc.tile_pool(name="expt", bufs=8) as epool,
            tc.tile_pool(name="tail", bufs=1) as tpool,
            tc.tile_pool(name="dram", bufs=1, space="DRAM") as dpool,
            tc.tile_pool(name="ps_scores", bufs=2, space="PSUM") as ps_s,
            tc.tile_pool(name="ps_ut", bufs=1, space="PSUM") as ps_ut,
            tc.tile_pool(name="ps_sums", bufs=1, space="PSUM") as ps_sum,
        ):
            # ---- stage own shard + weights into DRAM, AllGather ----
            # natural-layout shard: fp16 -> SBUF -> f32r -> local DRAM -> AG
            xs_sb_h = cpool.tile([128, SB, D], f16, tag="xs_sb_h", name="xs_sb_h")
            nc.sync.dma_start(
                xs_sb_h[:], xs_h[:].rearrange("(a p) d -> p a d", p=128))
            xs_sb = cpool.tile([128, SB, D], f32r, tag="xs_sb", name="xs_sb")
            nc.vector.tensor_copy(xs_sb[:], xs_sb_h[:])
            xs_int = dpool.tile([NL, D], f32r, tag="xs_int", name="xs_int")
            nc.sync.dma_start(
                xs_int[:].rearrange("(a p) d -> p a d", p=128), xs_sb[:])
            xg = dpool.tile([N, D], f32r, tag="xg", name="xg",
                            addr_space="Shared")
            nc.gpsimd.collective_compute(
                "AllGather", mybir.AluOpType.bypass, replica_groups=RG,
                ins=[xs_int.opt()], outs=[xg.opt()])

            # transposed shard via XBAR dma-transpose: fp16 -> f32r -> AG
            xsT_sb = []
            xsT_int = dpool.tile([D, NL], f32r, tag="xsT_int", name="xsT_int")
            for h in range(2):
                trh = cpool.tile([128, NL], f16, tag=f"trh{h}", name=f"trh{h}")
                nc.sync.dma_start(
                    trh[:], xs_h[:, h * 128:(h + 1) * 128], transpose=True)
                trf = cpool.tile([128, NL], f32r, tag=f"trf{h}", name=f"trf{h}")
                nc.vector.tensor_copy(trf[:], trh[:])
                nc.sync.dma_start(xsT_int[h * 128:(h + 1) * 128, :], trf[:])
                xsT_sb.append(trf)
            xgT = dpool.tile([P * D, NL], f32r, tag="xgT", name="xgT",
                             addr_space="Shared")
            nc.gpsimd.collective_compute(
                "AllGather", mybir.AluOpType.bypass, replica_groups=RG,
                ins=[xsT_int.opt()], outs=[xgT.opt()])

            # packed weights [WQ; WK.T; WV]: shard -> AG -> SBUF f32r tiles
            w_sb_h = cpool.tile([WSH, D], f16, tag="w_sb_h", name="w_sb_h")
            nc.sync.dma_start(w_sb_h[:], w_h[:])
            w_int = dpool.tile([WSH, D], f16, tag="w_int", name="w_int")
            nc.sync.dma_start(w_int[:], w_sb_h[:])
            w_all = dpool.tile([3 * D, D], f16, tag="w_all", name="w_all",
                               addr_space="Shared")
            nc.gpsimd.collective_compute(
                "AllGather", mybir.AluOpType.bypass, replica_groups=RG,
                ins=[w_int.opt()], outs=[w_all.opt()])

            def wtiles(base, nm):
                out = []
                for h in range(2):
                    th = cpool.tile([128, D], f16, tag=f"{nm}h{h}",
                                    name=f"{nm}h{h}")
                    nc.sync.dma_start(
                        th[:], w_all[base + h * 128: base + (h + 1) * 128, :])
                    tf = cpool.tile([128, D], f32r, tag=f"{nm}{h}",
                                    name=f"{nm}{h}")
                    nc.vector.tensor_copy(tf[:], th[:])
                    out.append(tf)
                return out

            wq_t = wtiles(0, "wq")
            wkt_t = wtiles(D, "wkt")
            wv_t = wtiles(2 * D, "wv")

            # ---- constants ----
            ones_col = cpool.tile([128, 1], f32r, tag="ones_col", name="ones_col")
            ones_row = cpool.tile([1, 128], f32r, tag="ones_row", name="ones_row")
            ones_f32 = cpool.tile([128, 1], f32, tag="ones_f32", name="ones_f32")
            ones_f32r = cpool.tile([1, 128], f32, tag="ones_f32r", name="ones_f32r")
            bias_t = cpool.tile([128, 1], f32, tag="bias_t", name="bias_t")
            nc.vector.memset(ones_f32[:], 1.0)
            nc.vector.memset(ones_f32r[:], 1.0)
            nc.vector.tensor_copy(ones_col[:], ones_f32[:])
            nc.vector.tensor_copy(ones_row[:], ones_f32r[:])
            nc.vector.memset(bias_t[:], EXP_SHIFT)

            # ---- qT = WQ.T @ xT_local ; M = WK @ qT ----
            qT_t = [ppool.tile([128, NL], f32r, tag=f"qt{h}", name=f"qt{h}") for h in range(2)]
            m_t = [ppool.tile([128, NL], f32r, tag=f"m{h}", name=f"m{h}") for h in range(2)]
            for dst, lhs in ((qT_t, wq_t), (m_t, wkt_t)):
                src = xsT_sb if dst is qT_t else qT_t
                for mh in range(2):
                    for nh in range(2):
                        pp = ps_s.tile([128, 512], f32, tag="scores", name="scores")
                        for kp in range(2):
                            nc.tensor.matmul(
                                pp[:],
                                lhs[kp][:, mh * 128:(mh + 1) * 128],
                                src[kp][:, nh * 512:(nh + 1) * 512],
                                start=(kp == 0), stop=(kp == 1),
                            )
                        nc.vector.tensor_copy(
                            dst[mh][:, nh * 512:(nh + 1) * 512], pp[:])

            # ---- persistent accumulators ----
            ut_ps = [ps_ut.tile([128, NL], f32, tag=f"ut{h}", name=f"ut{h}") for h in range(2)]
            sums_ps = [ps_sum.tile([1, 512], f32, tag=f"sums{h}", name=f"sums{h}")
                       for h in range(2)]

            # ---- main k-loop ----
            for sb in range(N // (KC * SB)):
                xt_t = [xtpool.tile([128, KC * SB], f32r, tag=f"xt{h}", name=f"xt{h}")
                        for h in range(2)]
                for h in range(2):
                    nc.sync.dma_start(
                        xt_t[h][:],
                        xgT[sb * 2 * 128 + h * 128:sb * 2 * 128 + (h + 1) * 128,
                            :])
                xn_t = xnpool.tile([128, SB, D], f32r, tag="xn", name="xn")
                nc.sync.dma_start(
                    xn_t[:],
                    xg[sb * KC * SB:(sb + 1) * KC * SB, :]
                    .rearrange("(a p) d -> p a d", p=128))

                for j in range(SB):
                    c = sb * SB + j
                    first, last = (c == 0), (c == NCHUNK - 1)
                    exps = []
                    for qh in range(2):
                        sp = ps_s.tile([128, 512], f32, tag="scores", name="scores")
                        for kp in range(2):
                            nc.tensor.matmul(
                                sp[:],
                                xt_t[kp][:, j * KC:(j + 1) * KC],
                                m_t[kp][:, qh * 512:(qh + 1) * 512],
                                start=(kp == 0), stop=(kp == 1),
                            )
                        et = epool.tile([128, 512], f32r, tag="expt", name="expt")
                        nc.scalar.activation(et[:], sp[:], EXP, bias=bias_t[:])
                        exps.append(et)
                    for qh in range(2):
                        et = exps[qh]
                        nc.tensor.matmul(
                            sums_ps[qh][:], ones_col[:], et[:],
                            start=first, stop=last)
                        for dh in range(2):
                            nc.tensor.matmul(
                                ut_ps[dh][:, qh * 512:(qh + 1) * 512],
                                xn_t[:, j, dh * 128:(dh + 1) * 128],
                                et[:],
                                start=first, stop=last)

            # ---- tail: softmax normalize + WV projection ----
            sums_sb = tpool.tile([1, NL], f32, tag="sums_sb", name="sums_sb")
            for qh in range(2):
                nc.vector.tensor_copy(
                    sums_sb[:, qh * 512:(qh + 1) * 512], sums_ps[qh][:])
            recip_sb = tpool.tile([1, NL], f32r, tag="recip_sb", name="recip_sb")
            with nc.allow_low_precision(reason="f32r is 4-byte, same mantissa path"):
                nc.vector.reciprocal(recip_sb[:], sums_sb[:])

            rb_sb = tpool.tile([128, NL], f32, tag="rb_sb", name="rb_sb")
            for qh in range(2):
                rp = ps_s.tile([128, 512], f32, tag="scores", name="scores")
                nc.tensor.matmul(
                    rp[:], ones_row[:],
                    recip_sb[:, qh * 512:(qh + 1) * 512],
                    start=True, stop=True)
                nc.vector.tensor_copy(rb_sb[:, qh * 512:(qh + 1) * 512], rp[:])

            utn_sb = [tpool.tile([128, NL], f32r, tag=f"utn{h}", name=f"utn{h}")
                      for h in range(2)]
            for dh in range(2):
                nc.vector.tensor_mul(utn_sb[dh][:], ut_ps[dh][:], rb_sb[:])

            # WV projection into two live PSUM tiles, then int8-quantize with a
            # per-core absmax scale (int8 + f32 scale halves the output bytes;
            # quantization error ~m/240 is ~4e-3 of the rel-err denominator)
            o_f32 = []
            am = tpool.tile([128, 2], f32, tag="am", name="am")
            for mh in range(2):
                op = ps_ut.tile([128, NL], f32, tag=f"ut{mh}", name=f"ut{mh}")
                for nh in range(2):
                    for kp in range(2):
                        nc.tensor.matmul(
                            op[:, nh * 512:(nh + 1) * 512],
                            wv_t[kp][:, mh * 128:(mh + 1) * 128],
                            utn_sb[kp][:, nh * 512:(nh + 1) * 512],
                            start=(kp == 0), stop=(kp == 1),
                        )
                of = tpool.tile([128, NL], f32, tag=f"of{mh}", name=f"of{mh}")
                nc.vector.tensor_copy(of[:], op[:])
                nc.vector.reduce_max(
                    am[:, mh:mh + 1], of[:], axis=mybir.AxisListType.X,
                    apply_absolute_value=True)
                o_f32.append(of)
            amax = tpool.tile([128, 1], f32, tag="amax", name="amax")
            nc.vector.reduce_max(amax[:], am[:], axis=mybir.AxisListType.X)
            nc.gpsimd.partition_all_reduce(
                amax[:], amax[:], channels=128,
                reduce_op=bass_isa.ReduceOp.absmax)
            sc126 = tpool.tile([128, 1], f32, tag="sc126", name="sc126")
            nc.scalar.activation(sc126[:], amax[:], COPY, scale=1.0 / QDEN)
            rcp = tpool.tile([128, 1], f32, tag="rcp", name="rcp")
            nc.vector.reciprocal(rcp[:], sc126[:])
            o_sb = [tpool.tile([128, NL], i8, tag=f"osb{h}", name=f"osb{h}") for h in range(2)]
            for mh in range(2):
                with nc.allow_low_precision(reason="int8 output transport"):
                    nc.vector.tensor_scalar_mul(o_sb[mh][:], o_f32[mh][:],
                                                rcp[:])
                # gpsimd queue, NOT sync: late sync-queue stores corrupt the
                # payload in this build (32-bit words get an fp32-style
                # low-12-bit rounding); the gpsimd DGE ring is clean.
                nc.gpsimd.dma_start(
                    outT[mh * 128:(mh + 1) * 128, :], o_sb[mh][:])
            nc.gpsimd.dma_start(om[:], amax[0:1, 0:1])

    nc.compile()
    return nc


def _setup_jax_cache():
    """Persistent XLA compilation cache: run_bass_kernel_spmd re-jits a fresh
    closure every call, so without this each call pays ~100ms of XLA
    recompile for the identical HLO."""
    if "jaxcache" in _CACHE:
        return
    import jax

    jax.config.update("jax_compilation_cache_dir", "/tmp/jaxcache")
    jax.config.update("jax_persistent_cache_min_entry_size_bytes", 0)
    jax.config.update("jax_persistent_cache_min_compile_time_secs", 0)
    _CACHE["jaxcache"] = True


def _get_nc():
    if "nc" not in _CACHE:
        _setup_jax_cache()
        _CACHE["nc"] = _build()
    return _CACHE["nc"]


def make_in_maps(input, WQ, WK, WV):
    """Per-core input maps: own fp16 x shard + 1/8 of packed [WQ; WK.T; WV]."""
    xh = np.ascontiguousarray(input, dtype=np.float32).astype(np.float16)
    wpack = np.concatenate(
        [np.asarray(WQ, dtype=np.float32),
         np.asarray(WK, dtype=np.float32).T,
         np.asarray(WV, dtype=np.float32)], axis=0).astype(np.float16)
    return [{
        "xs_h": np.ascontiguousarray(xh[c * NL:(c + 1) * NL]),
        "w_h": np.ascontiguousarray(wpack[c * WSH:(c + 1) * WSH]),
    } for c in range(P)]


def kernel(input, WQ, WK, WV):
    from concourse import bass_utils

    nc = _get_nc()
    in_maps = make_in_maps(input, WQ, WK, WV)
    res = bass_utils.run_bass_kernel_spmd(nc, in_maps, core_ids=list(range(P)))
    out = np.empty((N, D), dtype=np.float32)
    for c in range(P):
        scale = float(res.results[c]["om"][0, 0]) / QDEN
        out[c * NL:(c + 1) * NL, :] = (
            res.results[c]["outT"].astype(np.float32) * scale).T
    return out


# revision 23
# speedup vs baseline: 14.7945x; 1.2439x over previous
"""Sequence-parallel self-attention kernel for 8 TRN2 NeuronCores.

Reference computation (N=8192, D=256, fp32):
    q = x @ WQ; k = x @ WK; v = x @ WV
    out = softmax(q @ k.T) @ v

Host->device traffic is the wall-clock bottleneck (axon tunnel ~35 MB/s), so
each core receives ONLY its own fp16 shard plus a 1/8 slice of the packed
weights (~0.55 MB/core instead of 17.8 MB/core replicated), and the full x is
reconstructed on-device with AllGathers over NeuronLink (~14 us each):

  per core c (one fused fp16 input array xw_h [1120, 256]):
    rows 0..1023     own x rows (natural layout)
    rows 1024..1119  rows c*96..(c+1)*96 of packed [WQ; WK.T; WV]
  on device:
    AG#1: cast(xs_h)->f32r, gather -> xg  [8192, 256]   (natural x)
    AG#2: XBAR dma-transpose(xs_h)->f32r, gather -> xgT [2048, 1024]
          (8 stacked [256,1024] per-core transposed shards)
    AG#3: gather w_h -> w_all [768, 256] fp16, cast -> f32r weight tiles

Per-core algebra (identical to the proven replicated-input kernel; everything
stays transposed so softmax's k-reduction is a partition-axis ones-matmul):
    qT = WQ.T @ xT_local                      [256, 1024]
    M  = WK @ qT        (lhsT = WK.T)         [256, 1024]
    per k-chunk c (64 chunks of 128):
      scoresT = x_c @ M                       [128, 1024]   (= (q @ k.T).T chunk)
      expT    = exp(scoresT - 15)             (constant shift cancels in softmax)
      sums   += ones[128,1].T @ expT          [1, 1024]     (softmax denominator)
      UT     += x_c.T @ expT                  [256, 1024]   (= (attn_unnorm @ x).T)
    UTn  = UT * broadcast(1/sums)
    outT = WV.T @ UTn                         [256, 1024]   (int8 + f32 absmax
                                                             scale; host
                                                             dequantizes + .T)

All matmuls run as float32r (full PE rate at free-dim >= 256). fp16 input
transport + f32r compute + int8 output quantization land at ~5.7e-3 rel err
-- 3.5x under the 2e-2 gate.

Hardware quirk found while tuning: late SBUF->DRAM stores issued on the sync
DMA queue corrupt their payload in this build (every 32-bit word of some 4KB
spans gets +0x800 added then its low 12 bits cleared -- an fp32-mantissa-style
rounding). The output stores therefore go through the gpsimd DMA queue, which
is unaffected.
"""

import numpy as np

N, D, P = 8192, 256, 8
NL = N // P          # 1024 q-rows per core
KC = 128             # k-chunk size (contraction tile)
NCHUNK = N // KC     # 64
SB = 8               # k-chunks per DMA superblock
WSH = 3 * D // P     # 96 packed-weight rows per core
EXP_SHIFT = -15.0    # exp(s - 15): keeps ACT exp-table args in a good range
QDEN = 120.0         # int8 quant denominator; headroom vs 127 absorbs the
                     # ~1% error of the DVE reciprocal so +max never wraps

_CACHE = {}


def _build():
    import concourse.bacc as bacc
    import concourse.mybir as mybir
    import concourse.tile as tile

    import concourse.bass_isa as bass_isa

    f32 = mybir.dt.float32
    f32r = mybir.dt.float32r
    f16 = mybir.dt.float16
    i8 = mybir.dt.int8
    EXP = mybir.ActivationFunctionType.Exp
    COPY = mybir.ActivationFunctionType.Copy
    RG = [list(range(P))]

    nc = bacc.Bacc("TRN2", target_bir_lowering=False, debug=False,
                   enable_asserts=False, num_devices=P)

    xw_h = nc.dram_tensor("xw_h", [NL + WSH, D], f16, kind="ExternalInput").ap()
    xs_h = xw_h[0:NL, :]
    w_hs = xw_h[NL:NL + WSH, :]
    outT = nc.dram_tensor("outT", [D + 1, NL], i8, kind="ExternalOutput").ap()

    with tile.TileContext(nc) as tc:
        with (
            tc.tile_pool(name="const", bufs=1) as cpool,
            tc.tile_pool(name="proj", bufs=1) as ppool,
            tc.tile_pool(name="xts", bufs=4) as xtpool,
            tc.tile_pool(name="xns", bufs=4) as xnpool,
            tc.tile_pool(name="expt", bufs=8) as epool,
            tc.tile_pool(name="tail", bufs=1) as tpool,
            tc.tile_pool(name="dram", bufs=1, space="DRAM") as dpool,
            tc.tile_pool(name="ps_scores", bufs=2, space="PSUM") as ps_s,
            tc.tile_pool(name="ps_ut", bufs=1, space="PSUM") as ps_ut,
            tc.tile_pool(name="ps_sums", bufs=1, space="PSUM") as ps_sum,
        ):
            # ---- stage own shard + weights into DRAM, AllGather ----
            # natural-layout shard: fp16 -> SBUF -> f32r -> local DRAM -> AG
            xs_sb_h = cpool.tile([128, SB, D], f16, tag="xs_sb_h", name="xs_sb_h")
            nc.sync.dma_start(
                xs_sb_h[:], xs_h[:].rearrange("(a p) d -> p a d", p=128))
            xs_sb = cpool.tile([128, SB, D], f32r, tag="xs_sb", name="xs_sb")
            nc.vector.tensor_copy(xs_sb[:], xs_sb_h[:])
            xs_int = dpool.tile([NL, D], f32r, tag="xs_int", name="xs_int")
            nc.sync.dma_start(
                xs_int[:].rearrange("(a p) d -> p a d", p=128), xs_sb[:])
            xg = dpool.tile([N, D], f32r, tag="xg", name="xg",
                            addr_space="Shared")
            nc.gpsimd.collective_compute(
                "AllGather", mybir.AluOpType.bypass, replica_groups=RG,
                ins=[xs_int.opt()], outs=[xg.opt()])

            # transposed shard via XBAR dma-transpose: fp16 -> f32r -> AG
            xsT_sb = []
            xsT_int = dpool.tile([D, NL], f32r, tag="xsT_int", name="xsT_int")
            for h in range(2):
                trh = cpool.tile([128, NL], f16, tag=f"trh{h}", name=f"trh{h}")
                nc.sync.dma_start(
                    trh[:], xs_h[:, h * 128:(h + 1) * 128], transpose=True)
                trf = cpool.tile([128, NL], f32r, tag=f"trf{h}", name=f"trf{h}")
                nc.vector.tensor_copy(trf[:], trh[:])
                nc.sync.dma_start(xsT_int[h * 128:(h + 1) * 128, :], trf[:])
                xsT_sb.append(trf)
            xgT = dpool.tile([P * D, NL], f32r, tag="xgT", name="xgT",
                             addr_space="Shared")
            nc.gpsimd.collective_compute(
                "AllGather", mybir.AluOpType.bypass, replica_groups=RG,
                ins=[xsT_int.opt()], outs=[xgT.opt()])

            # packed weights [WQ; WK.T; WV]: shard -> AG -> SBUF f32r tiles
            w_sb_h = cpool.tile([WSH, D], f16, tag="w_sb_h", name="w_sb_h")
            nc.sync.dma_start(w_sb_h[:], w_hs)
            w_int = dpool.tile([WSH, D], f16, tag="w_int", name="w_int")
            nc.sync.dma_start(w_int[:], w_sb_h[:])
            w_all = dpool.tile([3 * D, D], f16, tag="w_all", name="w_all",
                               addr_space="Shared")
            nc.gpsimd.collective_compute(
                "AllGather", mybir.AluOpType.bypass, replica_groups=RG,
                ins=[w_int.opt()], outs=[w_all.opt()])

            def wtiles(base, nm):
                out = []
                for h in range(2):
                    th = cpool.tile([128, D], f16, tag=f"{nm}h{h}",
                                    name=f"{nm}h{h}")
                    nc.sync.dma_start(
                        th[:], w_all[base + h * 128: base + (h + 1) * 128, :])
                    tf = cpool.tile([128, D], f32r, tag=f"{nm}{h}",
                                    name=f"{nm}{h}")
                    nc.vector.tensor_copy(tf[:], th[:])
                    out.append(tf)
                return out

            wq_t = wtiles(0, "wq")
            wkt_t = wtiles(D, "wkt")
            wv_t = wtiles(2 * D, "wv")

            # ---- constants ----
            ones_col = cpool.tile([128, 1], f32r, tag="ones_col", name="ones_col")
            ones_row = cpool.tile([1, 128], f32r, tag="ones_row", name="ones_row")
            ones_f32 = cpool.tile([128, 1], f32, tag="ones_f32", name="ones_f32")
            ones_f32r = cpool.tile([1, 128], f32, tag="ones_f32r", name="ones_f32r")
            bias_t = cpool.tile([128, 1], f32, tag="bias_t", name="bias_t")
            nc.vector.memset(ones_f32[:], 1.0)
            nc.vector.memset(ones_f32r[:], 1.0)
            nc.vector.tensor_copy(ones_col[:], ones_f32[:])
            nc.vector.tensor_copy(ones_row[:], ones_f32r[:])
            nc.vector.memset(bias_t[:], EXP_SHIFT)

            # ---- qT = WQ.T @ xT_local ; M = WK @ qT ----
            qT_t = [ppool.tile([128, NL], f32r, tag=f"qt{h}", name=f"qt{h}") for h in range(2)]
            m_t = [ppool.tile([128, NL], f32r, tag=f"m{h}", name=f"m{h}") for h in range(2)]
            for dst, lhs in ((qT_t, wq_t), (m_t, wkt_t)):
                src = xsT_sb if dst is qT_t else qT_t
                for mh in range(2):
                    for nh in range(2):
                        pp = ps_s.tile([128, 512], f32, tag="scores", name="scores")
                        for kp in range(2):
                            nc.tensor.matmul(
                                pp[:],
                                lhs[kp][:, mh * 128:(mh + 1) * 128],
                                src[kp][:, nh * 512:(nh + 1) * 512],
                                start=(kp == 0), stop=(kp == 1),
                            )
                        nc.vector.tensor_copy(
                            dst[mh][:, nh * 512:(nh + 1) * 512], pp[:])

            # ---- persistent accumulators ----
            ut_ps = [ps_ut.tile([128, NL], f32, tag=f"ut{h}", name=f"ut{h}") for h in range(2)]
            sums_ps = [ps_sum.tile([1, 512], f32, tag=f"sums{h}", name=f"sums{h}")
                       for h in range(2)]

            # ---- main k-loop ----
            for sb in range(N // (KC * SB)):
                xt_t = [xtpool.tile([128, KC * SB], f32r, tag=f"xt{h}", name=f"xt{h}")
                        for h in range(2)]
                for h in range(2):
                    nc.sync.dma_start(
                        xt_t[h][:],
                        xgT[sb * 2 * 128 + h * 128:sb * 2 * 128 + (h + 1) * 128,
                            :])
                xn_t = xnpool.tile([128, SB, D], f32r, tag="xn", name="xn")
                nc.sync.dma_start(
                    xn_t[:],
                    xg[sb * KC * SB:(sb + 1) * KC * SB, :]
                    .rearrange("(a p) d -> p a d", p=128))

                for j in range(SB):
                    c = sb * SB + j
                    first, last = (c == 0), (c == NCHUNK - 1)
                    exps = []
                    for qh in range(2):
                        sp = ps_s.tile([128, 512], f32, tag="scores", name="scores")
                        for kp in range(2):
                            nc.tensor.matmul(
                                sp[:],
                                xt_t[kp][:, j * KC:(j + 1) * KC],
                                m_t[kp][:, qh * 512:(qh + 1) * 512],
                                start=(kp == 0), stop=(kp == 1),
                            )
                        et = epool.tile([128, 512], f32r, tag="expt", name="expt")
                        nc.scalar.activation(et[:], sp[:], EXP, bias=bias_t[:])
                        exps.append(et)
                    for qh in range(2):
                        et = exps[qh]
                        nc.tensor.matmul(
                            sums_ps[qh][:], ones_col[:], et[:],
                            start=first, stop=last)
                        for dh in range(2):
                            nc.tensor.matmul(
                                ut_ps[dh][:, qh * 512:(qh + 1) * 512],
                                xn_t[:, j, dh * 128:(dh + 1) * 128],
                                et[:],
                                start=first, stop=last)

            # ---- tail: softmax normalize + WV projection ----
            sums_sb = tpool.tile([1, NL], f32, tag="sums_sb", name="sums_sb")
            for qh in range(2):
                nc.vector.tensor_copy(
                    sums_sb[:, qh * 512:(qh + 1) * 512], sums_ps[qh][:])
            recip_sb = tpool.tile([1, NL], f32r, tag="recip_sb", name="recip_sb")
            with nc.allow_low_precision(reason="f32r is 4-byte, same mantissa path"):
                nc.vector.reciprocal(recip_sb[:], sums_sb[:])

            rb_sb = tpool.tile([128, NL], f32, tag="rb_sb", name="rb_sb")
            for qh in range(2):
                rp = ps_s.tile([128, 512], f32, tag="scores", name="scores")
                nc.tensor.matmul(
                    rp[:], ones_row[:],
                    recip_sb[:, qh * 512:(qh + 1) * 512],
                    start=True, stop=True)
                nc.vector.tensor_copy(rb_sb[:, qh * 512:(qh + 1) * 512], rp[:])

            utn_sb = [tpool.tile([128, NL], f32r, tag=f"utn{h}", name=f"utn{h}")
                      for h in range(2)]
            for dh in range(2):
                nc.vector.tensor_mul(utn_sb[dh][:], ut_ps[dh][:], rb_sb[:])

            # WV projection into two live PSUM tiles, then int8-quantize with a
            # per-core absmax scale (int8 + f32 scale halves the output bytes;
            # quantization error ~m/240 is ~4e-3 of the rel-err denominator)
            o_f32 = []
            am = tpool.tile([128, 2], f32, tag="am", name="am")
            for mh in range(2):
                op = ps_ut.tile([128, NL], f32, tag=f"ut{mh}", name=f"ut{mh}")
                for nh in range(2):
                    for kp in range(2):
                        nc.tensor.matmul(
                            op[:, nh * 512:(nh + 1) * 512],
                            wv_t[kp][:, mh * 128:(mh + 1) * 128],
                            utn_sb[kp][:, nh * 512:(nh + 1) * 512],
                            start=(kp == 0), stop=(kp == 1),
                        )
                of = tpool.tile([128, NL], f32, tag=f"of{mh}", name=f"of{mh}")
                nc.vector.tensor_copy(of[:], op[:])
                nc.vector.reduce_max(
                    am[:, mh:mh + 1], of[:], axis=mybir.AxisListType.X,
                    apply_absolute_value=True)
                o_f32.append(of)
            amax = tpool.tile([128, 1], f32, tag="amax", name="amax")
            nc.vector.reduce_max(amax[:], am[:], axis=mybir.AxisListType.X)
            nc.gpsimd.partition_all_reduce(
                amax[:], amax[:], channels=128,
                reduce_op=bass_isa.ReduceOp.absmax)
            sc126 = tpool.tile([128, 1], f32, tag="sc126", name="sc126")
            nc.scalar.activation(sc126[:], amax[:], COPY, scale=1.0 / QDEN)
            rcp = tpool.tile([128, 1], f32, tag="rcp", name="rcp")
            nc.vector.reciprocal(rcp[:], sc126[:])
            o_sb = [tpool.tile([128, NL], i8, tag=f"osb{h}", name=f"osb{h}") for h in range(2)]
            for mh in range(2):
                with nc.allow_low_precision(reason="int8 output transport"):
                    nc.vector.tensor_scalar_mul(o_sb[mh][:], o_f32[mh][:],
                                                rcp[:])
                # gpsimd queue, NOT sync: late sync-queue stores corrupt the
                # payload in this build (32-bit words get an fp32-style
                # low-12-bit rounding); the gpsimd DGE ring is clean.
                nc.gpsimd.dma_start(
                    outT[mh * 128:(mh + 1) * 128, :], o_sb[mh][:])
            # absmax f32 bitcast to 4 bytes, packed into outT's extra row
            nc.gpsimd.dma_start(outT[D:D + 1, 0:4],
                                amax[0:1, 0:1].bitcast(i8))

    nc.compile()
    return nc


def _setup_jax_cache():
    """Persistent XLA compilation cache: run_bass_kernel_spmd re-jits a fresh
    closure every call, so without this each call pays ~100ms of XLA
    recompile for the identical HLO."""
    if "jaxcache" in _CACHE:
        return
    import jax

    jax.config.update("jax_compilation_cache_dir", "/tmp/jaxcache")
    jax.config.update("jax_persistent_cache_min_entry_size_bytes", 0)
    jax.config.update("jax_persistent_cache_min_compile_time_secs", 0)
    _CACHE["jaxcache"] = True


def _get_nc():
    if "nc" not in _CACHE:
        _setup_jax_cache()
        _CACHE["nc"] = _build()
    return _CACHE["nc"]


def make_in_maps(input, WQ, WK, WV):
    """Per-core input maps: own fp16 x shard + 1/8 of packed [WQ; WK.T; WV],
    fused into one array (fewer tunnel transfers)."""
    xh = np.ascontiguousarray(input, dtype=np.float32).astype(np.float16)
    wpack = np.concatenate(
        [np.asarray(WQ, dtype=np.float32),
         np.asarray(WK, dtype=np.float32).T,
         np.asarray(WV, dtype=np.float32)], axis=0).astype(np.float16)
    return [{
        "xw_h": np.concatenate(
            [xh[c * NL:(c + 1) * NL], wpack[c * WSH:(c + 1) * WSH]], axis=0),
    } for c in range(P)]


def kernel(input, WQ, WK, WV):
    from concourse import bass_utils

    nc = _get_nc()
    in_maps = make_in_maps(input, WQ, WK, WV)
    res = bass_utils.run_bass_kernel_spmd(nc, in_maps, core_ids=list(range(P)))
    out = np.empty((N, D), dtype=np.float32)
    for c in range(P):
        o = res.results[c]["outT"]
        amax = np.frombuffer(o[D, 0:4].tobytes(), np.float32)[0]
        out[c * NL:(c + 1) * NL, :] = (
            o[:D].astype(np.float32) * (float(amax) / QDEN)).T
    return out


# revision 24
# speedup vs baseline: 15.9844x; 1.0804x over previous
"""Sequence-parallel self-attention kernel for 8 TRN2 NeuronCores.

Reference computation (N=8192, D=256, fp32):
    q = x @ WQ; k = x @ WK; v = x @ WV
    out = softmax(q @ k.T) @ v

Host->device traffic is the wall-clock bottleneck (axon tunnel ~35 MB/s), so
each core receives ONLY its own fp16 shard plus a 1/8 slice of the packed
weights (~0.55 MB/core instead of 17.8 MB/core replicated), and the full x is
reconstructed on-device with AllGathers over NeuronLink (~14 us each):

  per core c (one fused fp16 input array xw_h [1120, 256]):
    rows 0..1023     own x rows (natural layout)
    rows 1024..1119  rows c*96..(c+1)*96 of packed [WQ; WK.T; WV]
  on device:
    AG#1: cast(xs_h)->f32r, gather -> xg  [8192, 256]   (natural x)
    AG#2: XBAR dma-transpose(xs_h)->f32r, gather -> xgT [2048, 1024]
          (8 stacked [256,1024] per-core transposed shards)
    AG#3: gather w_h -> w_all [768, 256] fp16, cast -> f32r weight tiles

Per-core algebra (identical to the proven replicated-input kernel; everything
stays transposed so softmax's k-reduction is a partition-axis ones-matmul):
    qT = WQ.T @ xT_local                      [256, 1024]
    M  = WK @ qT        (lhsT = WK.T)         [256, 1024]
    per k-chunk c (64 chunks of 128):
      scoresT = x_c @ M                       [128, 1024]   (= (q @ k.T).T chunk)
      expT    = exp(scoresT - 15)             (constant shift cancels in softmax)
      sums   += ones[128,1].T @ expT          [1, 1024]     (softmax denominator)
      UT     += x_c.T @ expT                  [256, 1024]   (= (attn_unnorm @ x).T)
    UTn  = UT * broadcast(1/sums)
    outT = WV.T @ UTn                         [256, 1024]   (int8 + f32 absmax
                                                             scale; host
                                                             dequantizes + .T)

All matmuls run as float32r (full PE rate at free-dim >= 256). fp16 input
transport + f32r compute + int8 output quantization land at ~5.7e-3 rel err
-- 3.5x under the 2e-2 gate.

Hardware quirk found while tuning: late SBUF->DRAM stores issued on the sync
DMA queue corrupt their payload in this build (every 32-bit word of some 4KB
spans gets +0x800 added then its low 12 bits cleared -- an fp32-mantissa-style
rounding). The output stores therefore go through the gpsimd DMA queue, which
is unaffected.
"""

import numpy as np

N, D, P = 8192, 256, 8
NL = N // P          # 1024 q-rows per core
KC = 128             # k-chunk size (contraction tile)
NCHUNK = N // KC     # 64
SB = 8               # k-chunks per DMA superblock
WSH = 3 * D // P     # 96 packed-weight rows per core
EXP_SHIFT = -15.0    # exp(s - 15): keeps ACT exp-table args in a good range
QDEN = 120.0         # int8 quant denominator; headroom vs 127 absorbs the
                     # ~1% error of the DVE reciprocal so +max never wraps

_CACHE = {}


def _build():
    import concourse.bacc as bacc
    import concourse.mybir as mybir
    import concourse.tile as tile

    import concourse.bass_isa as bass_isa

    f32 = mybir.dt.float32
    f32r = mybir.dt.float32r
    f16 = mybir.dt.float16
    i8 = mybir.dt.int8
    EXP = mybir.ActivationFunctionType.Exp
    COPY = mybir.ActivationFunctionType.Copy
    RG = [list(range(P))]

    nc = bacc.Bacc("TRN2", target_bir_lowering=False, debug=False,
                   enable_asserts=False, num_devices=P,
                   enable_partition_id=False)

    xw_h = nc.dram_tensor("xw_h", [NL + WSH, D], f16, kind="ExternalInput").ap()
    xs_h = xw_h[0:NL, :]
    w_hs = xw_h[NL:NL + WSH, :]
    outT = nc.dram_tensor("outT", [D + 1, NL], i8, kind="ExternalOutput").ap()

    with tile.TileContext(nc) as tc:
        with (
            tc.tile_pool(name="const", bufs=1) as cpool,
            tc.tile_pool(name="proj", bufs=1) as ppool,
            tc.tile_pool(name="xts", bufs=4) as xtpool,
            tc.tile_pool(name="xns", bufs=4) as xnpool,
            tc.tile_pool(name="expt", bufs=8) as epool,
            tc.tile_pool(name="tail", bufs=1) as tpool,
            tc.tile_pool(name="dram", bufs=1, space="DRAM") as dpool,
            tc.tile_pool(name="ps_scores", bufs=2, space="PSUM") as ps_s,
            tc.tile_pool(name="ps_ut", bufs=1, space="PSUM") as ps_ut,
            tc.tile_pool(name="ps_sums", bufs=1, space="PSUM") as ps_sum,
        ):
            # ---- stage own shard + weights into DRAM, AllGather ----
            # natural-layout shard: fp16 -> SBUF -> f32r -> local DRAM -> AG
            xs_sb_h = cpool.tile([128, SB, D], f16, tag="xs_sb_h", name="xs_sb_h")
            nc.sync.dma_start(
                xs_sb_h[:], xs_h[:].rearrange("(a p) d -> p a d", p=128))
            xs_sb = cpool.tile([128, SB, D], f32r, tag="xs_sb", name="xs_sb")
            nc.vector.tensor_copy(xs_sb[:], xs_sb_h[:])
            xs_int = dpool.tile([NL, D], f32r, tag="xs_int", name="xs_int")
            nc.sync.dma_start(
                xs_int[:].rearrange("(a p) d -> p a d", p=128), xs_sb[:])
            xg = dpool.tile([N, D], f32r, tag="xg", name="xg",
                            addr_space="Shared")
            nc.gpsimd.collective_compute(
                "AllGather", mybir.AluOpType.bypass, replica_groups=RG,
                ins=[xs_int.opt()], outs=[xg.opt()])

            # transposed shard via XBAR dma-transpose: fp16 -> f32r -> AG
            xsT_sb = []
            xsT_int = dpool.tile([D, NL], f32r, tag="xsT_int", name="xsT_int")
            for h in range(2):
                trh = cpool.tile([128, NL], f16, tag=f"trh{h}", name=f"trh{h}")
                nc.sync.dma_start(
                    trh[:], xs_h[:, h * 128:(h + 1) * 128], transpose=True)
                trf = cpool.tile([128, NL], f32r, tag=f"trf{h}", name=f"trf{h}")
                nc.vector.tensor_copy(trf[:], trh[:])
                nc.sync.dma_start(xsT_int[h * 128:(h + 1) * 128, :], trf[:])
                xsT_sb.append(trf)
            xgT = dpool.tile([P * D, NL], f32r, tag="xgT", name="xgT",
                             addr_space="Shared")
            nc.gpsimd.collective_compute(
                "AllGather", mybir.AluOpType.bypass, replica_groups=RG,
                ins=[xsT_int.opt()], outs=[xgT.opt()])

            # packed weights [WQ; WK.T; WV]: shard -> AG -> SBUF f32r tiles
            w_sb_h = cpool.tile([WSH, D], f16, tag="w_sb_h", name="w_sb_h")
            nc.sync.dma_start(w_sb_h[:], w_hs)
            w_int = dpool.tile([WSH, D], f16, tag="w_int", name="w_int")
            nc.sync.dma_start(w_int[:], w_sb_h[:])
            w_all = dpool.tile([3 * D, D], f16, tag="w_all", name="w_all",
                               addr_space="Shared")
            nc.gpsimd.collective_compute(
                "AllGather", mybir.AluOpType.bypass, replica_groups=RG,
                ins=[w_int.opt()], outs=[w_all.opt()])

            def wtiles(base, nm):
                out = []
                for h in range(2):
                    th = cpool.tile([128, D], f16, tag=f"{nm}h{h}",
                                    name=f"{nm}h{h}")
                    nc.sync.dma_start(
                        th[:], w_all[base + h * 128: base + (h + 1) * 128, :])
                    tf = cpool.tile([128, D], f32r, tag=f"{nm}{h}",
                                    name=f"{nm}{h}")
                    nc.vector.tensor_copy(tf[:], th[:])
                    out.append(tf)
                return out

            wq_t = wtiles(0, "wq")
            wkt_t = wtiles(D, "wkt")
            wv_t = wtiles(2 * D, "wv")

            # ---- constants ----
            ones_col = cpool.tile([128, 1], f32r, tag="ones_col", name="ones_col")
            ones_row = cpool.tile([1, 128], f32r, tag="ones_row", name="ones_row")
            ones_f32 = cpool.tile([128, 1], f32, tag="ones_f32", name="ones_f32")
            ones_f32r = cpool.tile([1, 128], f32, tag="ones_f32r", name="ones_f32r")
            bias_t = cpool.tile([128, 1], f32, tag="bias_t", name="bias_t")
            nc.vector.memset(ones_f32[:], 1.0)
            nc.vector.memset(ones_f32r[:], 1.0)
            nc.vector.tensor_copy(ones_col[:], ones_f32[:])
            nc.vector.tensor_copy(ones_row[:], ones_f32r[:])
            nc.vector.memset(bias_t[:], EXP_SHIFT)

            # ---- qT = WQ.T @ xT_local ; M = WK @ qT ----
            qT_t = [ppool.tile([128, NL], f32r, tag=f"qt{h}", name=f"qt{h}") for h in range(2)]
            m_t = [ppool.tile([128, NL], f32r, tag=f"m{h}", name=f"m{h}") for h in range(2)]
            for dst, lhs in ((qT_t, wq_t), (m_t, wkt_t)):
                src = xsT_sb if dst is qT_t else qT_t
                for mh in range(2):
                    for nh in range(2):
                        pp = ps_s.tile([128, 512], f32, tag="scores", name="scores")
                        for kp in range(2):
                            nc.tensor.matmul(
                                pp[:],
                                lhs[kp][:, mh * 128:(mh + 1) * 128],
                                src[kp][:, nh * 512:(nh + 1) * 512],
                                start=(kp == 0), stop=(kp == 1),
                            )
                        nc.vector.tensor_copy(
                            dst[mh][:, nh * 512:(nh + 1) * 512], pp[:])

            # ---- persistent accumulators ----
            ut_ps = [ps_ut.tile([128, NL], f32, tag=f"ut{h}", name=f"ut{h}") for h in range(2)]
            sums_ps = [ps_sum.tile([1, 512], f32, tag=f"sums{h}", name=f"sums{h}")
                       for h in range(2)]

            # ---- main k-loop ----
            for sb in range(N // (KC * SB)):
                xt_t = [xtpool.tile([128, KC * SB], f32r, tag=f"xt{h}", name=f"xt{h}")
                        for h in range(2)]
                for h in range(2):
                    nc.sync.dma_start(
                        xt_t[h][:],
                        xgT[sb * 2 * 128 + h * 128:sb * 2 * 128 + (h + 1) * 128,
                            :])
                xn_t = xnpool.tile([128, SB, D], f32r, tag="xn", name="xn")
                nc.sync.dma_start(
                    xn_t[:],
                    xg[sb * KC * SB:(sb + 1) * KC * SB, :]
                    .rearrange("(a p) d -> p a d", p=128))

                for j in range(SB):
                    c = sb * SB + j
                    first, last = (c == 0), (c == NCHUNK - 1)
                    exps = []
                    for qh in range(2):
                        sp = ps_s.tile([128, 512], f32, tag="scores", name="scores")
                        for kp in range(2):
                            nc.tensor.matmul(
                                sp[:],
                                xt_t[kp][:, j * KC:(j + 1) * KC],
                                m_t[kp][:, qh * 512:(qh + 1) * 512],
                                start=(kp == 0), stop=(kp == 1),
                            )
                        et = epool.tile([128, 512], f32r, tag="expt", name="expt")
                        nc.scalar.activation(et[:], sp[:], EXP, bias=bias_t[:])
                        exps.append(et)
                    for qh in range(2):
                        et = exps[qh]
                        nc.tensor.matmul(
                            sums_ps[qh][:], ones_col[:], et[:],
                            start=first, stop=last)
                        for dh in range(2):
                            nc.tensor.matmul(
                                ut_ps[dh][:, qh * 512:(qh + 1) * 512],
                                xn_t[:, j, dh * 128:(dh + 1) * 128],
                                et[:],
                                start=first, stop=last)

            # ---- tail: softmax normalize + WV projection ----
            sums_sb = tpool.tile([1, NL], f32, tag="sums_sb", name="sums_sb")
            for qh in range(2):
                nc.vector.tensor_copy(
                    sums_sb[:, qh * 512:(qh + 1) * 512], sums_ps[qh][:])
            recip_sb = tpool.tile([1, NL], f32r, tag="recip_sb", name="recip_sb")
            with nc.allow_low_precision(reason="f32r is 4-byte, same mantissa path"):
                nc.vector.reciprocal(recip_sb[:], sums_sb[:])

            rb_sb = tpool.tile([128, NL], f32, tag="rb_sb", name="rb_sb")
            for qh in range(2):
                rp = ps_s.tile([128, 512], f32, tag="scores", name="scores")
                nc.tensor.matmul(
                    rp[:], ones_row[:],
                    recip_sb[:, qh * 512:(qh + 1) * 512],
                    start=True, stop=True)
                nc.vector.tensor_copy(rb_sb[:, qh * 512:(qh + 1) * 512], rp[:])

            utn_sb = [tpool.tile([128, NL], f32r, tag=f"utn{h}", name=f"utn{h}")
                      for h in range(2)]
            for dh in range(2):
                nc.vector.tensor_mul(utn_sb[dh][:], ut_ps[dh][:], rb_sb[:])

            # WV projection into two live PSUM tiles, then int8-quantize with a
            # per-core absmax scale (int8 + f32 scale halves the output bytes;
            # quantization error ~m/240 is ~4e-3 of the rel-err denominator)
            o_f32 = []
            am = tpool.tile([128, 2], f32, tag="am", name="am")
            for mh in range(2):
                op = ps_ut.tile([128, NL], f32, tag=f"ut{mh}", name=f"ut{mh}")
                for nh in range(2):
                    for kp in range(2):
                        nc.tensor.matmul(
                            op[:, nh * 512:(nh + 1) * 512],
                            wv_t[kp][:, mh * 128:(mh + 1) * 128],
                            utn_sb[kp][:, nh * 512:(nh + 1) * 512],
                            start=(kp == 0), stop=(kp == 1),
                        )
                of = tpool.tile([128, NL], f32, tag=f"of{mh}", name=f"of{mh}")
                nc.vector.tensor_copy(of[:], op[:])
                nc.vector.reduce_max(
                    am[:, mh:mh + 1], of[:], axis=mybir.AxisListType.X,
                    apply_absolute_value=True)
                o_f32.append(of)
            amax = tpool.tile([128, 1], f32, tag="amax", name="amax")
            nc.vector.reduce_max(amax[:], am[:], axis=mybir.AxisListType.X)
            nc.gpsimd.partition_all_reduce(
                amax[:], amax[:], channels=128,
                reduce_op=bass_isa.ReduceOp.absmax)
            sc126 = tpool.tile([128, 1], f32, tag="sc126", name="sc126")
            nc.scalar.activation(sc126[:], amax[:], COPY, scale=1.0 / QDEN)
            rcp = tpool.tile([128, 1], f32, tag="rcp", name="rcp")
            nc.vector.reciprocal(rcp[:], sc126[:])
            o_sb = [tpool.tile([128, NL], i8, tag=f"osb{h}", name=f"osb{h}") for h in range(2)]
            for mh in range(2):
                with nc.allow_low_precision(reason="int8 output transport"):
                    nc.vector.tensor_scalar_mul(o_sb[mh][:], o_f32[mh][:],
                                                rcp[:])
                # gpsimd queue, NOT sync: late sync-queue stores corrupt the
                # payload in this build (32-bit words get an fp32-style
                # low-12-bit rounding); the gpsimd DGE ring is clean.
                nc.gpsimd.dma_start(
                    outT[mh * 128:(mh + 1) * 128, :], o_sb[mh][:])
            # absmax f32 bitcast to 4 bytes, packed into outT's extra row
            nc.gpsimd.dma_start(outT[D:D + 1, 0:4],
                                amax[0:1, 0:1].bitcast(i8))

    nc.compile()
    return nc


def _setup_jax_cache():
    """Persistent XLA compilation cache: run_bass_kernel_spmd re-jits a fresh
    closure every call, so without this each call pays ~100ms of XLA
    recompile for the identical HLO."""
    if "jaxcache" in _CACHE:
        return
    import jax

    jax.config.update("jax_compilation_cache_dir", "/tmp/jaxcache")
    jax.config.update("jax_persistent_cache_min_entry_size_bytes", 0)
    jax.config.update("jax_persistent_cache_min_compile_time_secs", 0)
    _CACHE["jaxcache"] = True


def _get_nc():
    if "nc" not in _CACHE:
        _setup_jax_cache()
        _CACHE["nc"] = _build()
    return _CACHE["nc"]


def make_in_maps(input, WQ, WK, WV):
    """Per-core input maps: own fp16 x shard + 1/8 of packed [WQ; WK.T; WV],
    fused into one array (fewer tunnel transfers)."""
    xh = np.ascontiguousarray(input, dtype=np.float32).astype(np.float16)
    wpack = np.concatenate(
        [np.asarray(WQ, dtype=np.float32),
         np.asarray(WK, dtype=np.float32).T,
         np.asarray(WV, dtype=np.float32)], axis=0).astype(np.float16)
    return [{
        "xw_h": np.concatenate(
            [xh[c * NL:(c + 1) * NL], wpack[c * WSH:(c + 1) * WSH]], axis=0),
    } for c in range(P)]


def kernel(input, WQ, WK, WV):
    from concourse import bass_utils

    nc = _get_nc()
    in_maps = make_in_maps(input, WQ, WK, WV)
    res = bass_utils.run_bass_kernel_spmd(nc, in_maps, core_ids=list(range(P)))
    out = np.empty((N, D), dtype=np.float32)
    for c in range(P):
        o = res.results[c]["outT"]
        amax = np.frombuffer(o[D, 0:4].tobytes(), np.float32)[0]
        out[c * NL:(c + 1) * NL, :] = (
            o[:D].astype(np.float32) * (float(amax) / QDEN)).T
    return out
